# revision 11
# baseline (speedup 1.0000x reference)
"""Distributed multi-head attention for TRN2 (8 NeuronCores).

Problem: b=2, t=2048, d=1024, h=16 heads, head_dim=64.
  out = softmax((q Wq^T)(k Wk^T)^T / 8) (v Wv^T) Wo^T + bo   (per head)

Sharding: core c -> batch i_b = c//4, head group i_h = c%4 (4 heads = 256
features). Each core projects Q/K/V for its batch+heads, runs attention,
then an 8-core AllToAll reshards head-major -> time-major so each core
computes the final projection for its 512-row time slice.

The schedule is built around the ScalarE exp stream (the hard floor:
16.8M exp elements/core at 1 elem/cycle/lane = ~130us). Scores for a
head PAIR are computed with two row-tiled K=64 matmuls (tile_position
(0,0)/(64,0) via operand base partitions) into the two banks of one
[128,1024] PSUM tile, so a single ACTIVATE covers both heads' exp for a
512-query quarter. Queries are processed in 512-col quarters so the
per-head P@V accumulators (with the softmax-denominator ones-row, M=65)
need only 1 bank each. PSUM: s2 double-buffer 4 banks + po 2 banks +
2-bank work pool used to JIT the K/Q/V projections and the output
projection inside the attention loop's PE slack.

Device layouts (no on-chip transposes): activations streamed as X^T
[d, t]; K/Q kept transposed, pair-packed [128=2x64 hn, t]; scores as S^T
[t_k, t_q]; softmax denominator via an extra ones-column in the V
operand; output projection computes Y^T [d, t_slice].

Datapath fp16 (host-converted); PSUM accumulation fp32.

The AllToAll runs over all 8 cores (4-core groups are unsupported):
shards are duplicated to both batch groups and the final projection uses
16 virtual hn-chunks whose weights are host-side zero-masked for the
chunks belonging to the other batch (SPMD rank-independence). Even
chunks (pair 0) are pre-accumulated right after the first AllToAll and
stashed in SBUF with the bias folded in; the tail after the second
AllToAll only runs the 8 odd-chunk accumulations + one DVE add each.
"""

import numpy as np

import concourse.bass as bass
import concourse.mybir as mybir
import concourse.tile as tile
from concourse import bacc
from concourse.bass_utils import run_bass_kernel_spmd

N_CORES = 8
B = 2
T = 2048
D = 1024
HEADS = 16
HD = 64
HPC = 4            # heads per core
HN = HPC * HD      # 256 head-features per core
TS = T // 4        # 512 time-slice per core after reshard
NQ = T // 512      # 4 query quarters
f32 = mybir.dt.float32
f16 = mybir.dt.float16
EXP = mybir.ActivationFunctionType.Exp

_cached = None


def _build():
    nc = bacc.Bacc("TRN2", target_bir_lowering=False, debug=False,
                   num_devices=N_CORES)

    xqT = nc.dram_tensor("xqT", [D, T], f16, kind="ExternalInput")
    xkT = nc.dram_tensor("xkT", [D, T], f16, kind="ExternalInput")
    xvT = nc.dram_tensor("xvT", [D, T], f16, kind="ExternalInput")
    wqT = nc.dram_tensor("wqT", [D, HN], f16, kind="ExternalInput")
    wkT = nc.dram_tensor("wkT", [D, HN], f16, kind="ExternalInput")
    wvT = nc.dram_tensor("wvT", [D, HN], f16, kind="ExternalInput")
    woT = nc.dram_tensor("woT", [2 * D, D], f16, kind="ExternalInput")
    bo = nc.dram_tensor("bo", [D, 1], f32, kind="ExternalInput")
    out = nc.dram_tensor("out", [D, TS], f32, kind="ExternalOutput")

    onesv_d = nc.inline_tensor(np.ones((128, 64), np.float16), name="onesv_c")

    with tile.TileContext(nc) as tc:
        with (
            tc.tile_pool(name="bigp", bufs=40) as bigp,       # x chunks / wo
            tc.tile_pool(name="ep", bufs=5) as ep,            # exp outputs
            tc.tile_pool(name="s2p", bufs=2, space="PSUM") as s2p,
            tc.tile_pool(name="pop", bufs=1, space="PSUM") as pop,
            tc.tile_pool(name="wkp", bufs=2, space="PSUM") as wkp,
            tc.tile_pool(name="otp", bufs=2) as otp,          # OT / L16 / lr32
            tc.tile_pool(name="rp", bufs=1) as rp,            # a2a_out chunks
            tc.tile_pool(name="ypp", bufs=2) as ypp,          # outproj partials
            tc.tile_pool(name="dram", bufs=1, space="DRAM") as dram,
            tc.tile_pool(name="pers", bufs=1) as pers,
        ):
            # ---- persistent SBUF ----
            KT = [pers.tile([128, T], f16, tag=f"KT{p}", name=f"KT{p}")
                  for p in range(2)]
            QP = [pers.tile([128, T], f16, tag=f"QP{p}", name=f"QP{p}")
                  for p in range(2)]
            V = pers.tile([128, 16, HPC, HD + 1], f16, tag="Vsb", name="Vsb")
            onesf = pers.tile([128, HD], f16, tag="onesf", name="onesf")
            bo_sb = pers.tile([128, 8], f32, tag="bo_sb", name="bo_sb")
            wk_sb = pers.tile([128, 8, HN], f16, tag="wk_sb", name="wk_sb")
            wq_sb = pers.tile([128, 8, HN], f16, tag="wq_sb", name="wq_sb")
            wv_sb = pers.tile([128, 8, HN], f16, tag="wv_sb", name="wv_sb")

            nc.sync.dma_start(onesf[:], onesv_d.ap())
            nc.sync.dma_start(V[:, :, :, HD:HD + 1], onesv_d.ap())
            for dd in range(8):
                nc.sync.dma_start(
                    bo_sb[:, dd:dd + 1], bo[dd * 128:(dd + 1) * 128, 0:1]
                )
            nc.sync.dma_start(
                wk_sb[:], wkT[:].rearrange("(c p) n -> p c n", p=128))
            nc.sync.dma_start(
                wq_sb[:], wqT[:].rearrange("(c p) n -> p c n", p=128))
            nc.sync.dma_start(
                wv_sb[:], wvT[:].rearrange("(c p) n -> p c n", p=128))

            # tiny dummy AllToAll first: pays the mesh-algorithm pipeline
            # fill (~12us/rank staged wave) under the lead-in/attention
            dmy_in = dram.tile([8, 16, 64], f16, name="dmy_in")
            dmy_out = dram.tile([8, 16, 64], f16, name="dmy_out")
            nc.sync.dma_start(
                dmy_in[:].rearrange("s p n -> (s p) n"), onesv_d.ap().bitcast(f16)
            )
            nc.gpsimd.collective_compute(
                "AllToAll",
                mybir.AluOpType.bypass,
                replica_groups=[list(range(N_CORES))],
                ins=[dmy_in.opt()],
                outs=[dmy_out.opt()],
            )

            a2a_in = [
                dram.tile([8, 128, TS], f16, name=f"a2a_in{p}") for p in range(2)
            ]
            a2a_out = [
                dram.tile([8, 128, TS], f16, name=f"a2a_out{p}") for p in range(2)
            ]

            # ---- input staging: x chunks [128, 1024] (kk, col-half) ----
            xs = {}

            def stage_x(xdram, key, kk, ch):
                t0 = bigp.tile([128, 1024], f16, tag="big", name=f"x{key}{kk}{ch}")
                nc.sync.dma_start(t0[:], xdram[kk * 128:(kk + 1) * 128,
                                               ch * 1024:(ch + 1) * 1024])
                xs[(key, kk, ch)] = t0

            def xsl(key, kk, cols):
                # slice [128, 512-ish] of staged chunk for given col range
                ch, off = cols // 1024, cols % 1024
                return xs[(key, kk, ch)][:, off:off + 512]

            # lead-in critical DMAs: xk/xq first column-halves
            for kk in range(8):
                stage_x(xkT, "k", kk, 0)
            for kk in range(8):
                stage_x(xqT, "q", kk, 0)
            for kk in range(8):
                stage_x(xvT, "v", kk, 0)

            # ---- projection emitters (JIT-interleaved) ----
            def proj_kq(dest, w_sb, key, p, cols):
                # dest[p][:, cols:cols+512] <- pair-packed projection
                ps = wkp.tile([128, 512], f32, tag="wk", name="prj")
                for kk in range(8):
                    nc.tensor.matmul(
                        ps[:],
                        w_sb[:, kk, p * 128:(p + 1) * 128],
                        xsl(key, kk, cols),
                        start=(kk == 0), stop=(kk == 7),
                    )
                nc.vector.tensor_copy(dest[p][:, cols:cols + 512], ps[:])

            def proj_v(tt):
                # V for key-tile tt (all 4 heads), N=256
                ps = wkp.tile([128, 512], f32, tag="wk", name="pv")
                ch, off = (tt * 128) // 1024, (tt * 128) % 1024
                for kk in range(8):
                    nc.tensor.matmul(
                        ps[:, 0:HN],
                        xs[("v", kk, ch)][:, off:off + 128],
                        wv_sb[:, kk, :],
                        start=(kk == 0), stop=(kk == 7),
                    )
                nc.vector.tensor_copy(
                    V[:, tt, :, 0:HD],
                    ps[:, 0:HN].rearrange("p (h n) -> p h n", h=HPC),
                )

            # ---- attention ----
            def normalize_a(p, qq, po, h):
                # DVE part: reciprocal + denominators row + OT cast
                lr32 = otp.tile([HD + 1, 512], f32, tag="lr", name="lr32")
                ot = otp.tile([HD, 512], f16, tag=f"ot{h}", name="ot")
                lrow = otp.tile([HD + 1, 512], f16, tag="lrow", name="lrow")
                with nc.allow_low_precision(reason="fp16 datapath by design"):
                    nc.vector.reciprocal_approx_fast(lr32[:], po[0:HD + 1, :])
                    nc.vector.tensor_copy(
                        lrow[HD:HD + 1, :], lr32[HD:HD + 1, :])
                    nc.vector.tensor_copy(ot[:], po[0:HD, :])
                return ot, lrow

            def normalize_b(p, qq, h, ot, lrow):
                # PE broadcast of 1/den + DVE mult + a2a staging
                pb = wkp.tile([HD, 512], f32, tag="wk", name="pb")
                nc.tensor.matmul(
                    pb[:],
                    onesf[HD:HD + 1, 0:HD],
                    lrow[HD:HD + 1, :],
                    start=True, stop=True,
                )
                with nc.allow_low_precision(reason="fp16 datapath by design"):
                    nc.vector.tensor_tensor(
                        ot[:], ot[:], pb[:], op=mybir.AluOpType.mult)
                for rep in (0, 4):
                    nc.sync.dma_start(
                        a2a_in[p][qq + rep, h * HD:(h + 1) * HD, :], ot[:])

            def a2a(p):
                nc.gpsimd.collective_compute(
                    "AllToAll",
                    mybir.AluOpType.bypass,
                    replica_groups=[list(range(N_CORES))],
                    ins=[a2a_in[p].opt()],
                    outs=[a2a_out[p].opt()],
                )

            # ---- output projection helpers ----
            wo_sb = {}
            rt_sb = {}
            ypart = {}

            def load_rt(cc):
                pp, blk = cc % 2, cc // 2
                t0 = rp.tile([128, TS], f16, tag=f"rt{cc}", name=f"rt{cc}")
                nc.sync.dma_start(t0[:], a2a_out[pp][blk])
                rt_sb[cc] = t0

            def load_wo(cc):
                t0 = bigp.tile([128, 1024], f16, tag="big", name=f"wo{cc}")
                nc.sync.dma_start(t0[:], woT[cc * 128:(cc + 1) * 128, :])
                wo_sb[cc] = t0

            def outproj_even(dd):
                ps = wkp.tile([128, 512], f32, tag="wk", name=f"ye{dd}")
                for i, cc in enumerate(range(0, 16, 2)):
                    nc.tensor.matmul(
                        ps[:],
                        wo_sb[cc][:, dd * 128:(dd + 1) * 128],
                        rt_sb[cc][:],
                        start=(i == 0), stop=(i == 7),
                    )
                yp = ypp.tile([128, 512], f32, tag=f"yp{dd}", name=f"yp{dd}")
                nc.vector.tensor_scalar_add(yp[:], ps[:], bo_sb[:, dd:dd + 1])
                ypart[dd] = yp

            def outproj_odd(dd):
                ps = wkp.tile([128, 512], f32, tag="wk", name=f"yo{dd}")
                for i, cc in enumerate(range(1, 16, 2)):
                    nc.tensor.matmul(
                        ps[:],
                        wo_sb[cc][:, dd * 128:(dd + 1) * 128],
                        rt_sb[cc][:],
                        start=(i == 0), stop=(i == 7),
                    )
                y = ypp.tile([128, 512], f32, tag=f"yp{dd}", name=f"y{dd}")
                nc.vector.tensor_tensor(
                    y[:], ypart[dd][:], ps[:], op=mybir.AluOpType.add)
                nc.sync.dma_start(out[dd * 128:(dd + 1) * 128, :], y[:])

            # ---- lead-in projections ----
            proj_kq(KT, wk_sb, "k", 0, 0)      # K pair0, keys 0:512
            proj_kq(QP, wq_sb, "q", 0, 0)      # Q pair0, q 0:512
            proj_v(0)

            # deferred-work queue, consumed one item per tk slot
            def later_dma():
                for kk in range(8):
                    stage_x(xkT, "k", kk, 1)

            def later_dma2():
                for kk in range(8):
                    stage_x(xvT, "v", kk, 1)

            def later_dma3():
                for kk in range(8):
                    stage_x(xqT, "q", kk, 1)

            def dummy2():
                nc.gpsimd.collective_compute(
                    "AllToAll",
                    mybir.AluOpType.bypass,
                    replica_groups=[list(range(N_CORES))],
                    ins=[dmy_out.opt()],
                    outs=[dmy_in.opt()],
                )

            # build fill schedule: dict (p, qq, tk) -> list of callables
            fill = {}

            def add_fill(p, qq, tk, fn):
                fill.setdefault((p, qq, tk), []).append(fn)

            # (p0, qq0): V-proj JIT + K-pair0 key chunks + DMA continuations
            add_fill(0, 0, 0, later_dma)
            add_fill(0, 0, 0, lambda: proj_kq(KT, wk_sb, "k", 0, 512))
            for tt in range(1, 16):
                add_fill(0, 0, tt - 1, lambda tt=tt: proj_v(tt))
            add_fill(0, 0, 2, later_dma2)
            add_fill(0, 0, 4, lambda: proj_kq(KT, wk_sb, "k", 0, 1024))
            add_fill(0, 0, 5, later_dma3)
            add_fill(0, 0, 8, lambda: proj_kq(KT, wk_sb, "k", 0, 1536))
            add_fill(0, 0, 12, lambda: proj_kq(QP, wq_sb, "q", 0, 512))
            # (p0, qq1): Q0 rest + K1 start (tk>=4: clear of the deferred
            # normalize broadcasts that hold the work pool around tk2)
            add_fill(0, 1, 4, lambda: proj_kq(QP, wq_sb, "q", 0, 1024))
            add_fill(0, 1, 7, lambda: proj_kq(KT, wk_sb, "k", 1, 0))
            add_fill(0, 1, 11, lambda: proj_kq(KT, wk_sb, "k", 1, 512))
            add_fill(0, 1, 14, lambda: proj_kq(QP, wq_sb, "q", 0, 1536))
            # (p0, qq2): K1 rest + Q1 start + dummy2
            add_fill(0, 2, 4, lambda: proj_kq(KT, wk_sb, "k", 1, 1024))
            add_fill(0, 2, 6, dummy2)
            add_fill(0, 2, 8, lambda: proj_kq(KT, wk_sb, "k", 1, 1536))
            add_fill(0, 2, 11, lambda: proj_kq(QP, wq_sb, "q", 1, 0))
            # (p0, qq3): Q1 rest
            add_fill(0, 3, 4, lambda: proj_kq(QP, wq_sb, "q", 1, 512))
            add_fill(0, 3, 8, lambda: proj_kq(QP, wq_sb, "q", 1, 1024))
            add_fill(0, 3, 12, lambda: proj_kq(QP, wq_sb, "q", 1, 1536))
            # (p1, qq0): woT loads begin; rt evens after a2a(0) emission (tk2)
            for i, cc in enumerate(range(0, 16, 2)):
                add_fill(1, 0, 2 * i, lambda cc=cc: load_wo(cc))
                add_fill(1, 0, 3 + i, lambda cc=cc: load_rt(cc))
            # (p1, qq1..2): even outproj accumulation into SBUF partials
            for dd in range(8):
                qq, tk = 1 + dd // 4, (dd % 4) * 3 + 4
                add_fill(1, qq, tk, lambda dd=dd: outproj_even(dd))
            # (p1, qq3): odd wo loads
            for i, cc in enumerate(range(1, 16, 2)):
                add_fill(1, 3, 2 * i, lambda cc=cc: load_wo(cc))

            # ---- main attention loop ----
            norm_pending = []
            for p in range(2):
                for qq in range(NQ):
                    po = [pop.tile([HD + 1, 512], f32, tag=f"po{h}",
                                   name=f"po{p}{qq}{h}") for h in range(2)]
                    es = {}
                    for tk in range(16):
                        s2q = s2p.tile([128, 1024], f32, tag="s2", name="s2q")
                        for h in range(2):
                            nc.tensor.matmul(
                                s2q[:, h * 512:(h + 1) * 512],
                                KT[p][h * 64:(h + 1) * 64,
                                      tk * 128:(tk + 1) * 128],
                                QP[p][h * 64:(h + 1) * 64,
                                      qq * 512:(qq + 1) * 512],
                                start=True, stop=True,
                            )
                        e = ep.tile([128, 1024], f16, tag="e", name="e")
                        nc.scalar.activation(e[:], s2q[:], EXP, scale=0.125)
                        # deferred normalize_b of the previous quarter
                        if tk == 2 and norm_pending:
                            for fn in norm_pending:
                                fn()
                            norm_pending.clear()
                        if tk > 0:
                            for h in range(2):
                                nc.tensor.matmul(
                                    po[h][:],
                                    V[:, tk - 1, p * 2 + h, :],
                                    es[tk - 1][:, h * 512:(h + 1) * 512],
                                    start=(tk == 1), stop=False,
                                )
                        es[tk] = e
                        for fn in fill.get((p, qq, tk), []):
                            fn()
                    for h in range(2):
                        nc.tensor.matmul(
                            po[h][:],
                            V[:, 15, p * 2 + h, :],
                            es[15][:, h * 512:(h + 1) * 512],
                            start=False, stop=True,
                        )
                    # normalize: DVE part now (frees po), PE/bcast deferred
                    last = (p == 1 and qq == NQ - 1)
                    for h in range(2):
                        ot, lrow = normalize_a(p, qq, po[h], h)
                        norm_pending.append(
                            lambda p=p, qq=qq, h=h, ot=ot, lrow=lrow:
                            normalize_b(p, qq, h, ot, lrow))
                    if last:
                        for fn in norm_pending:
                            fn()
                        norm_pending.clear()
                # pair-0's last normalize_b is deferred into pair-1 qq0; the
                # collective must be EMITTED after those a2a_in DMAs or Tile
                # would order them behind the collective's read (WAR)
                if p == 0:
                    norm_pending.append(lambda: a2a(0))
                else:
                    a2a(1)

            # ---- tail: odd outproj chunks ----
            for i, cc in enumerate(range(1, 16, 2)):
                load_rt(cc)
            for dd in range(8):
                outproj_odd(dd)

    nc.compile()
    return nc


def _shard_inputs(k, q, v, Wk, Wq, Wv, Wo, bo):
    woT_full = np.ascontiguousarray(Wo.T).astype(np.float16)  # [hn, d]
    in_maps = []
    for c in range(N_CORES):
        i_b, i_h = c // 4, c % 4
        sl = slice(i_h * HN, (i_h + 1) * HN)
        # masked out-projection weights: 16 virtual chunks (cc = 2*blk + p)
        woT_m = np.zeros((2 * D, D), np.float16)
        for cc in range(16):
            p, blk = cc % 2, cc // 2
            if blk // 4 == i_b:
                ghc = 2 * (blk % 4) + p  # global hn chunk 0..7
                woT_m[cc * 128:(cc + 1) * 128, :] = \
                    woT_full[ghc * 128:(ghc + 1) * 128, :]
        in_maps.append({
            "xqT": q[i_b].T.astype(np.float16),
            "xkT": k[i_b].T.astype(np.float16),
            "xvT": v[i_b].T.astype(np.float16),
            "wqT": Wq[sl].T.astype(np.float16),
            "wkT": Wk[sl].T.astype(np.float16),
            "wvT": Wv[sl].T.astype(np.float16),
            "woT": woT_m,
            "bo": np.ascontiguousarray(bo.reshape(D, 1)).astype(np.float32),
        })
    return in_maps


def _run(in_maps, **kw):
    global _cached
    if _cached is None:
        _cached = _build()
    return run_bass_kernel_spmd(_cached, in_maps, core_ids=list(range(N_CORES)),
                                **kw)


def kernel(k, q, v, Wk, Wq, Wv, Wo, bo):
    k, q, v = (np.asarray(x, np.float32) for x in (k, q, v))
    Wk, Wq, Wv, Wo, bo = (np.asarray(x, np.float32) for x in (Wk, Wq, Wv, Wo, bo))
    in_maps = _shard_inputs(k, q, v, Wk, Wq, Wv, Wo, bo)
    res = _run(in_maps)
    out = np.empty((B, T, D), np.float32)
    for c in range(N_CORES):
        i_b, i_h = c // 4, c % 4
        out[i_b, i_h * TS:(i_h + 1) * TS, :] = res.results[c]["out"].T
    return out


# revision 19
# speedup vs baseline: 1.1231x; 1.1231x over previous
"""Distributed multi-head attention for TRN2 (8 NeuronCores).

Problem: b=2, t=2048, d=1024, h=16 heads, head_dim=64.
  out = softmax((q Wq^T)(k Wk^T)^T / 8) (v Wv^T) Wo^T + bo   (per head)

Sharding: core c -> batch i_b = c//4, head group i_h = c%4 (4 heads = 256
features). Each core projects Q/K/V for its batch+heads, runs attention,
then an 8-core AllToAll reshards head-major -> time-major so each core
computes the final projection for its 512-row time slice.

The schedule is built around the ScalarE exp stream (the hard floor:
16.8M exp elements/core at 1 elem/cycle/lane = ~130us). Scores for a
head PAIR are computed with two row-tiled K=64 matmuls (tile_position
(0,0)/(64,0) via operand base partitions) into the two banks of one
[128,1024] PSUM tile, so a single ACTIVATE covers both heads' exp for a
512-query quarter. Queries are processed in 512-col quarters so the
per-head P@V accumulators (with the softmax-denominator ones-row, M=65)
need only 1 bank each. PSUM: s2 double-buffer 4 banks + po 2 banks +
2-bank work pool used to JIT the K/Q/V projections and the output
projection inside the attention loop's PE slack.

Device layouts (no on-chip transposes): activations streamed as X^T
[d, t]; K/Q kept transposed, pair-packed [128=2x64 hn, t]; scores as S^T
[t_k, t_q]; softmax denominator via an extra ones-column in the V
operand; output projection computes Y^T [d, t_slice].

Datapath fp16 (host-converted); PSUM accumulation fp32.

The AllToAll runs over all 8 cores (4-core groups are unsupported):
shards are duplicated to both batch groups and the final projection uses
16 virtual hn-chunks whose weights are host-side zero-masked for the
chunks belonging to the other batch (SPMD rank-independence). Even
chunks (pair 0) are pre-accumulated right after the first AllToAll and
stashed in SBUF with the bias folded in; the tail after the second
AllToAll only runs the 8 odd-chunk accumulations + one DVE add each.
"""

import numpy as np

import concourse.bass as bass
import concourse.mybir as mybir
import concourse.tile as tile
from concourse import bacc
from concourse.bass_utils import run_bass_kernel_spmd

N_CORES = 8
B = 2
T = 2048
D = 1024
HEADS = 16
HD = 64
HPC = 4            # heads per core
HN = HPC * HD      # 256 head-features per core
TS = T // 4        # 512 time-slice per core after reshard
NQ = T // 512      # 4 query quarters
f32 = mybir.dt.float32
f16 = mybir.dt.float16
EXP = mybir.ActivationFunctionType.Exp

_cached = None


def _build():
    nc = bacc.Bacc("TRN2", target_bir_lowering=False, debug=False,
                   num_devices=N_CORES)

    xqT = nc.dram_tensor("xqT", [D, T], f16, kind="ExternalInput")
    xkT = nc.dram_tensor("xkT", [D, T], f16, kind="ExternalInput")
    xvT = nc.dram_tensor("xvT", [D, T], f16, kind="ExternalInput")
    wqT = nc.dram_tensor("wqT", [D, HN], f16, kind="ExternalInput")
    wkT = nc.dram_tensor("wkT", [D, HN], f16, kind="ExternalInput")
    wvT = nc.dram_tensor("wvT", [D, HN], f16, kind="ExternalInput")
    woT = nc.dram_tensor("woT", [2 * D, D], f16, kind="ExternalInput")
    bo = nc.dram_tensor("bo", [D, 1], f32, kind="ExternalInput")
    out = nc.dram_tensor("out", [D, TS], f32, kind="ExternalOutput")

    onesv_d = nc.inline_tensor(np.ones((128, 64), np.float16), name="onesv_c")

    with tile.TileContext(nc) as tc:
        with (
            tc.tile_pool(name="bigp", bufs=40) as bigp,       # x chunks / wo
            tc.tile_pool(name="ep", bufs=5) as ep,            # exp outputs
            tc.tile_pool(name="s2p", bufs=2, space="PSUM") as s2p,
            tc.tile_pool(name="pop", bufs=1, space="PSUM") as pop,
            tc.tile_pool(name="wkp", bufs=2, space="PSUM") as wkp,
            tc.tile_pool(name="otp", bufs=2) as otp,          # OT / L16 / lr32
            tc.tile_pool(name="rp", bufs=1) as rp,            # a2a_out chunks
            tc.tile_pool(name="ypp", bufs=2) as ypp,          # outproj partials
            tc.tile_pool(name="dram", bufs=1, space="DRAM") as dram,
            tc.tile_pool(name="pers", bufs=1) as pers,
        ):
            # ---- persistent SBUF ----
            KT = [pers.tile([128, T], f16, tag=f"KT{p}", name=f"KT{p}")
                  for p in range(2)]
            # per-head Q, zero-padded to K=128 so scores run full-array mode
            # (row-tiled K=64 scores force a TensorE drain on every switch
            # to/from the 128-row P@V/proj matmuls — measured net loss)
            QT = [pers.tile([128, T], f16, tag=f"QT{h}", name=f"QT{h}")
                  for h in range(HPC)]
            V = pers.tile([128, 16, HPC, HD + 1], f16, tag="Vsb", name="Vsb")
            onesf = pers.tile([128, HD], f16, tag="onesf", name="onesf")
            bo_sb = pers.tile([128, 8], f32, tag="bo_sb", name="bo_sb")
            wk_sb = pers.tile([128, 8, HN], f16, tag="wk_sb", name="wk_sb")
            wq_sb = pers.tile([128, 8, HN], f16, tag="wq_sb", name="wq_sb")
            wv_sb = pers.tile([128, 8, HN], f16, tag="wv_sb", name="wv_sb")

            nc.sync.dma_start(onesf[:], onesv_d.ap())
            # ones column of the V operand via memset (a DMA from the inline
            # tensor would issue 8192 two-byte descriptors and starve the
            # lead-in DMA queues)
            nc.vector.memset(V[:, :, :, HD:HD + 1], 1.0)
            # zero the unused K-half of each per-head Q operand
            for h in range(HPC):
                z0, z1 = ((HD, 128) if h % 2 == 0 else (0, HD))
                nc.vector.memset(QT[h][z0:z1, :], 0.0)
            for dd in range(8):
                nc.sync.dma_start(
                    bo_sb[:, dd:dd + 1], bo[dd * 128:(dd + 1) * 128, 0:1]
                )
            nc.sync.dma_start(
                wk_sb[:], wkT[:].rearrange("(c p) n -> p c n", p=128))
            nc.sync.dma_start(
                wq_sb[:], wqT[:].rearrange("(c p) n -> p c n", p=128))
            nc.sync.dma_start(
                wv_sb[:], wvT[:].rearrange("(c p) n -> p c n", p=128))

            # tiny dummy AllToAll first: pays the mesh-algorithm pipeline
            # fill (~12us/rank staged wave) under the lead-in/attention
            dmy_in = dram.tile([8, 16, 64], f16, name="dmy_in")
            dmy_out = dram.tile([8, 16, 64], f16, name="dmy_out")
            nc.sync.dma_start(
                dmy_in[:].rearrange("s p n -> (s p) n"), onesv_d.ap().bitcast(f16)
            )
            nc.gpsimd.collective_compute(
                "AllToAll",
                mybir.AluOpType.bypass,
                replica_groups=[list(range(N_CORES))],
                ins=[dmy_in.opt()],
                outs=[dmy_out.opt()],
            )

            a2a_in = [
                dram.tile([8, 128, TS], f16, name=f"a2a_in{p}") for p in range(2)
            ]
            a2a_out = [
                dram.tile([8, 128, TS], f16, name=f"a2a_out{p}") for p in range(2)
            ]

            # ---- input staging: x chunks [128, 1024] (kk, col-half) ----
            xs = {}

            def stage_x(xdram, key, kk, ch):
                t0 = bigp.tile([128, 1024], f16, tag="big", name=f"x{key}{kk}{ch}")
                nc.sync.dma_start(t0[:], xdram[kk * 128:(kk + 1) * 128,
                                               ch * 1024:(ch + 1) * 1024])
                xs[(key, kk, ch)] = t0

            def xsl(key, kk, cols):
                # slice [128, 512-ish] of staged chunk for given col range
                ch, off = cols // 1024, cols % 1024
                return xs[(key, kk, ch)][:, off:off + 512]

            # lead-in critical DMAs: xk/xq first column-halves
            for kk in range(8):
                stage_x(xkT, "k", kk, 0)
            for kk in range(8):
                stage_x(xqT, "q", kk, 0)
            for kk in range(8):
                stage_x(xvT, "v", kk, 0)

            # ---- projection emitters (JIT-interleaved) ----
            def _proj_ps(w_sb, key, p, cols):
                ps = wkp.tile([128, 512], f32, tag="wk", name="prj")
                for kk in range(8):
                    nc.tensor.matmul(
                        ps[:],
                        w_sb[:, kk, p * 128:(p + 1) * 128],
                        xsl(key, kk, cols),
                        start=(kk == 0), stop=(kk == 7),
                    )
                return ps

            def proj_k(p, cols):
                ps = _proj_ps(wk_sb, "k", p, cols)
                nc.vector.tensor_copy(KT[p][:, cols:cols + 512], ps[:])

            def proj_q(p, cols):
                # pair-packed psum -> per-head padded QT (head parity keeps
                # each head's hn rows at their in-pair partition offsets)
                ps = _proj_ps(wq_sb, "q", p, cols)
                sl = slice(cols, cols + 512)
                nc.vector.tensor_copy(QT[2 * p][0:HD, sl], ps[0:HD, :])
                nc.vector.tensor_copy(QT[2 * p + 1][HD:128, sl], ps[HD:128, :])

            def proj_v(tt):
                # V for key-tile tt (all 4 heads), N=256
                ps = wkp.tile([128, 512], f32, tag="wk", name="pv")
                ch, off = (tt * 128) // 1024, (tt * 128) % 1024
                for kk in range(8):
                    nc.tensor.matmul(
                        ps[:, 0:HN],
                        xs[("v", kk, ch)][:, off:off + 128],
                        wv_sb[:, kk, :],
                        start=(kk == 0), stop=(kk == 7),
                    )
                nc.vector.tensor_copy(
                    V[:, tt, :, 0:HD],
                    ps[:, 0:HN].rearrange("p (h n) -> p h n", h=HPC),
                )

            # ---- attention ----
            def normalize_a(p, qq, po, h):
                # DVE part: reciprocal + denominators row + OT cast
                lr32 = otp.tile([HD + 1, 512], f32, tag="lr", name="lr32")
                ot = otp.tile([HD, 512], f16, tag=f"ot{h}", name="ot")
                lrow = otp.tile([HD + 1, 512], f16, tag="lrow", name="lrow")
                with nc.allow_low_precision(reason="fp16 datapath by design"):
                    nc.vector.reciprocal_approx_fast(lr32[:], po[0:HD + 1, :])
                    nc.vector.tensor_copy(
                        lrow[HD:HD + 1, :], lr32[HD:HD + 1, :])
                    nc.vector.tensor_copy(ot[:], po[0:HD, :])
                return ot, lrow

            def normalize_b(p, qq, h, ot, lrow):
                # PE broadcast of 1/den + DVE mult + a2a staging
                pb = wkp.tile([HD, 512], f32, tag="wk", name="pb")
                nc.tensor.matmul(
                    pb[:],
                    onesf[HD:HD + 1, 0:HD],
                    lrow[HD:HD + 1, :],
                    start=True, stop=True,
                )
                with nc.allow_low_precision(reason="fp16 datapath by design"):
                    nc.vector.tensor_tensor(
                        ot[:], ot[:], pb[:], op=mybir.AluOpType.mult)
                for rep in (0, 4):
                    nc.sync.dma_start(
                        a2a_in[p][qq + rep, h * HD:(h + 1) * HD, :], ot[:])

            def a2a(p):
                nc.gpsimd.collective_compute(
                    "AllToAll",
                    mybir.AluOpType.bypass,
                    replica_groups=[list(range(N_CORES))],
                    ins=[a2a_in[p].opt()],
                    outs=[a2a_out[p].opt()],
                )

            # ---- output projection helpers ----
            wo_sb = {}
            rt_sb = {}
            ypart = {}

            def load_rt(cc):
                pp, blk = cc % 2, cc // 2
                t0 = rp.tile([128, TS], f16, tag=f"rt{cc}", name=f"rt{cc}")
                nc.sync.dma_start(t0[:], a2a_out[pp][blk])
                rt_sb[cc] = t0

            def load_wo(cc):
                t0 = bigp.tile([128, 1024], f16, tag="big", name=f"wo{cc}")
                nc.sync.dma_start(t0[:], woT[cc * 128:(cc + 1) * 128, :])
                wo_sb[cc] = t0

            def outproj_even(dd):
                ps = wkp.tile([128, 512], f32, tag="wk", name=f"ye{dd}")
                for i, cc in enumerate(range(0, 16, 2)):
                    nc.tensor.matmul(
                        ps[:],
                        wo_sb[cc][:, dd * 128:(dd + 1) * 128],
                        rt_sb[cc][:],
                        start=(i == 0), stop=(i == 7),
                    )
                yp = ypp.tile([128, 512], f32, tag=f"yp{dd}", name=f"yp{dd}")
                nc.vector.tensor_scalar_add(yp[:], ps[:], bo_sb[:, dd:dd + 1])
                ypart[dd] = yp

            def outproj_odd(dd):
                ps = wkp.tile([128, 512], f32, tag="wk", name=f"yo{dd}")
                for i, cc in enumerate(range(1, 16, 2)):
                    nc.tensor.matmul(
                        ps[:],
                        wo_sb[cc][:, dd * 128:(dd + 1) * 128],
                        rt_sb[cc][:],
                        start=(i == 0), stop=(i == 7),
                    )
                y = ypp.tile([128, 512], f32, tag=f"yp{dd}", name=f"y{dd}")
                nc.vector.tensor_tensor(
                    y[:], ypart[dd][:], ps[:], op=mybir.AluOpType.add)
                nc.sync.dma_start(out[dd * 128:(dd + 1) * 128, :], y[:])

            # ---- lead-in projections ----
            proj_k(0, 0)                       # K pair0, keys 0:512
            proj_q(0, 0)                       # Q pair0, q 0:512
            proj_v(0)

            # deferred-work queue, consumed one item per tk slot
            def later_dma():
                for kk in range(8):
                    stage_x(xkT, "k", kk, 1)

            def later_dma2():
                for kk in range(8):
                    stage_x(xvT, "v", kk, 1)

            def later_dma3():
                for kk in range(8):
                    stage_x(xqT, "q", kk, 1)

            # comms-warm drip: a real AllToAll issued after ~60us+ of comms
            # idle pays a ~25us re-establishment cost (vs ~5us warm), so a
            # tiny dummy fires every quarter, time-gated by DMAing the
            # current e tile as its input
            cur_e = [None]
            drip_i = [0]

            def drip():
                a, b = ((dmy_in, dmy_out) if drip_i[0] % 2 == 0
                        else (dmy_out, dmy_in))
                drip_i[0] += 1
                nc.sync.dma_start(
                    a[:].rearrange("s p n -> (s p) n"), cur_e[0][:, 0:64])
                nc.gpsimd.collective_compute(
                    "AllToAll",
                    mybir.AluOpType.bypass,
                    replica_groups=[list(range(N_CORES))],
                    ins=[a.opt()],
                    outs=[b.opt()],
                )

            # build fill schedule: dict (p, qq, tk) -> list of callables
            fill = {}

            def add_fill(p, qq, tk, fn):
                fill.setdefault((p, qq, tk), []).append(fn)

            # (p0, qq0): V-proj JIT + K-pair0 key chunks + DMA continuations
            add_fill(0, 0, 0, later_dma)
            add_fill(0, 0, 0, lambda: proj_k(0, 512))
            for tt in range(1, 16):
                add_fill(0, 0, tt - 1, lambda tt=tt: proj_v(tt))
            add_fill(0, 0, 2, later_dma2)
            add_fill(0, 0, 4, lambda: proj_k(0, 1024))
            add_fill(0, 0, 5, later_dma3)
            add_fill(0, 0, 8, lambda: proj_k(0, 1536))
            add_fill(0, 0, 12, lambda: proj_q(0, 512))
            # (p0, qq1): Q0 rest + K1 start (tk>=4: clear of the deferred
            # normalize broadcasts that hold the work pool around tk2)
            add_fill(0, 1, 4, lambda: proj_q(0, 1024))
            add_fill(0, 1, 7, lambda: proj_k(1, 0))
            add_fill(0, 1, 11, lambda: proj_k(1, 512))
            add_fill(0, 1, 14, lambda: proj_q(0, 1536))
            # (p0, qq2): K1 rest + Q1 start
            add_fill(0, 2, 4, lambda: proj_k(1, 1024))
            add_fill(0, 2, 8, lambda: proj_k(1, 1536))
            add_fill(0, 2, 11, lambda: proj_q(1, 0))
            # comms-warm drips, one per quarter
            for pq in ((0, 1), (0, 2), (0, 3), (1, 0), (1, 1), (1, 2)):
                add_fill(pq[0], pq[1], 9, drip)
            # (p0, qq3): Q1 rest
            add_fill(0, 3, 4, lambda: proj_q(1, 512))
            add_fill(0, 3, 8, lambda: proj_q(1, 1024))
            add_fill(0, 3, 12, lambda: proj_q(1, 1536))
            # (p1, qq0): woT loads begin; rt evens after a2a(0) emission (tk2)
            for i, cc in enumerate(range(0, 16, 2)):
                add_fill(1, 0, 2 * i, lambda cc=cc: load_wo(cc))
                add_fill(1, 0, 3 + i, lambda cc=cc: load_rt(cc))
            # (p1, qq1..2): even outproj accumulation into SBUF partials
            for dd in range(8):
                qq, tk = 1 + dd // 4, (dd % 4) * 3 + 4
                add_fill(1, qq, tk, lambda dd=dd: outproj_even(dd))
            # (p1, qq3): odd wo loads
            for i, cc in enumerate(range(1, 16, 2)):
                add_fill(1, 3, 2 * i, lambda cc=cc: load_wo(cc))

            # ---- main attention loop ----
            norm_pending = []
            for p in range(2):
                for qq in range(NQ):
                    po = [pop.tile([HD + 1, 512], f32, tag=f"po{h}",
                                   name=f"po{p}{qq}{h}") for h in range(2)]
                    es = {}
                    for tk in range(16):
                        s2q = s2p.tile([128, 1024], f32, tag="s2", name="s2q")
                        for h in range(2):
                            nc.tensor.matmul(
                                s2q[:, h * 512:(h + 1) * 512],
                                KT[p][:, tk * 128:(tk + 1) * 128],
                                QT[2 * p + h][:, qq * 512:(qq + 1) * 512],
                                start=True, stop=True,
                            )
                        e = ep.tile([128, 1024], f16, tag="e", name="e")
                        cur_e[0] = e
                        nc.scalar.activation(e[:], s2q[:], EXP, scale=0.125)
                        # deferred normalize_b of the previous quarter
                        if tk == 2 and norm_pending:
                            for fn in norm_pending:
                                fn()
                            norm_pending.clear()
                        if tk > 0:
                            for h in range(2):
                                nc.tensor.matmul(
                                    po[h][:],
                                    V[:, tk - 1, p * 2 + h, :],
                                    es[tk - 1][:, h * 512:(h + 1) * 512],
                                    start=(tk == 1), stop=False,
                                )
                        es[tk] = e
                        for fn in fill.get((p, qq, tk), []):
                            fn()
                    for h in range(2):
                        nc.tensor.matmul(
                            po[h][:],
                            V[:, 15, p * 2 + h, :],
                            es[15][:, h * 512:(h + 1) * 512],
                            start=False, stop=True,
                        )
                    # normalize: DVE part now (frees po), PE/bcast deferred
                    last = (p == 1 and qq == NQ - 1)
                    for h in range(2):
                        ot, lrow = normalize_a(p, qq, po[h], h)
                        norm_pending.append(
                            lambda p=p, qq=qq, h=h, ot=ot, lrow=lrow:
                            normalize_b(p, qq, h, ot, lrow))
                    if last:
                        for fn in norm_pending:
                            fn()
                        norm_pending.clear()
                # pair-0's last normalize_b is deferred into pair-1 qq0; the
                # collective must be EMITTED after those a2a_in DMAs or Tile
                # would order them behind the collective's read (WAR)
                if p == 0:
                    norm_pending.append(lambda: a2a(0))
                else:
                    a2a(1)

            # ---- tail: odd outproj chunks ----
            for i, cc in enumerate(range(1, 16, 2)):
                load_rt(cc)
            for dd in range(8):
                outproj_odd(dd)

    nc.compile()
    return nc


def _shard_inputs(k, q, v, Wk, Wq, Wv, Wo, bo):
    woT_full = np.ascontiguousarray(Wo.T).astype(np.float16)  # [hn, d]
    in_maps = []
    for c in range(N_CORES):
        i_b, i_h = c // 4, c % 4
        sl = slice(i_h * HN, (i_h + 1) * HN)
        # masked out-projection weights: 16 virtual chunks (cc = 2*blk + p)
        woT_m = np.zeros((2 * D, D), np.float16)
        for cc in range(16):
            p, blk = cc % 2, cc // 2
            if blk // 4 == i_b:
                ghc = 2 * (blk % 4) + p  # global hn chunk 0..7
                woT_m[cc * 128:(cc + 1) * 128, :] = \
                    woT_full[ghc * 128:(ghc + 1) * 128, :]
        in_maps.append({
            "xqT": q[i_b].T.astype(np.float16),
            "xkT": k[i_b].T.astype(np.float16),
            "xvT": v[i_b].T.astype(np.float16),
            "wqT": Wq[sl].T.astype(np.float16),
            "wkT": Wk[sl].T.astype(np.float16),
            "wvT": Wv[sl].T.astype(np.float16),
            "woT": woT_m,
            "bo": np.ascontiguousarray(bo.reshape(D, 1)).astype(np.float32),
        })
    return in_maps


def _run(in_maps, **kw):
    global _cached
    if _cached is None:
        _cached = _build()
    return run_bass_kernel_spmd(_cached, in_maps, core_ids=list(range(N_CORES)),
                                **kw)


def kernel(k, q, v, Wk, Wq, Wv, Wo, bo):
    k, q, v = (np.asarray(x, np.float32) for x in (k, q, v))
    Wk, Wq, Wv, Wo, bo = (np.asarray(x, np.float32) for x in (Wk, Wq, Wv, Wo, bo))
    in_maps = _shard_inputs(k, q, v, Wk, Wq, Wv, Wo, bo)
    res = _run(in_maps)
    out = np.empty((B, T, D), np.float32)
    for c in range(N_CORES):
        i_b, i_h = c // 4, c % 4
        out[i_b, i_h * TS:(i_h + 1) * TS, :] = res.results[c]["out"].T
    return out


# revision 28
# speedup vs baseline: 1.1551x; 1.0285x over previous
"""Distributed multi-head attention for TRN2 (8 NeuronCores).

Problem: b=2, t=2048, d=1024, h=16 heads, head_dim=64.
  out = softmax((q Wq^T)(k Wk^T)^T / 8) (v Wv^T) Wo^T + bo   (per head)

Sharding: core c -> batch i_b = c//4, head group i_h = c%4 (4 heads = 256
features). Each core projects Q/K/V for its batch+heads, runs attention,
then an 8-core AllToAll reshards head-major -> time-major so each core
computes the final projection for its 512-row time slice.

The schedule is built around the ScalarE exp stream (the hard floor:
16.8M exp elements/core at 1 elem/cycle/lane = ~130us). Scores for a
head PAIR are computed with two row-tiled K=64 matmuls (tile_position
(0,0)/(64,0) via operand base partitions) into the two banks of one
[128,1024] PSUM tile, so a single ACTIVATE covers both heads' exp for a
512-query quarter. Queries are processed in 512-col quarters so the
per-head P@V accumulators (with the softmax-denominator ones-row, M=65)
need only 1 bank each. PSUM: s2 double-buffer 4 banks + po 2 banks +
2-bank work pool used to JIT the K/Q/V projections and the output
projection inside the attention loop's PE slack.

Device layouts (no on-chip transposes): activations streamed as X^T
[d, t]; K/Q kept transposed, pair-packed [128=2x64 hn, t]; scores as S^T
[t_k, t_q]; softmax denominator via an extra ones-column in the V
operand; output projection computes Y^T [d, t_slice].

Datapath fp16 (host-converted); PSUM accumulation fp32.

The AllToAll runs over all 8 cores (4-core groups are unsupported):
shards are duplicated to both batch groups and the final projection uses
16 virtual hn-chunks whose weights are host-side zero-masked for the
chunks belonging to the other batch (SPMD rank-independence). Even
chunks (pair 0) are pre-accumulated right after the first AllToAll and
stashed in SBUF with the bias folded in; the tail after the second
AllToAll only runs the 8 odd-chunk accumulations + one DVE add each.
"""

import numpy as np

import concourse.bass as bass
import concourse.mybir as mybir
import concourse.tile as tile
from concourse import bacc
from concourse.bass_utils import run_bass_kernel_spmd

N_CORES = 8
B = 2
T = 2048
D = 1024
HEADS = 16
HD = 64
HPC = 4            # heads per core
HN = HPC * HD      # 256 head-features per core
TS = T // 4        # 512 time-slice per core after reshard
NQ = T // 512      # 4 query quarters
f32 = mybir.dt.float32
f16 = mybir.dt.float16
EXP = mybir.ActivationFunctionType.Exp

_cached = None


def _build():
    nc = bacc.Bacc("TRN2", target_bir_lowering=False, debug=False,
                   num_devices=N_CORES)

    xqT = nc.dram_tensor("xqT", [D, T], f16, kind="ExternalInput")
    xkT = nc.dram_tensor("xkT", [D, T], f16, kind="ExternalInput")
    xvT = nc.dram_tensor("xvT", [D, T], f16, kind="ExternalInput")
    wqT = nc.dram_tensor("wqT", [D, HN], f16, kind="ExternalInput")
    wkT = nc.dram_tensor("wkT", [D, HN], f16, kind="ExternalInput")
    wvT = nc.dram_tensor("wvT", [D, HN], f16, kind="ExternalInput")
    woT = nc.dram_tensor("woT", [D, D], f16, kind="ExternalInput")
    bo = nc.dram_tensor("bo", [D, 1], f32, kind="ExternalInput")
    out = nc.dram_tensor("out", [D, TS], f32, kind="ExternalOutput")

    onesv_d = nc.inline_tensor(np.ones((128, 64), np.float16), name="onesv_c")

    with tile.TileContext(nc) as tc:
        with (
            tc.tile_pool(name="bigp", bufs=40) as bigp,       # x chunks / wo
            tc.tile_pool(name="ep", bufs=5) as ep,            # exp outputs
            tc.tile_pool(name="s2p", bufs=2, space="PSUM") as s2p,
            tc.tile_pool(name="pop", bufs=1, space="PSUM") as pop,
            tc.tile_pool(name="wkp", bufs=2, space="PSUM") as wkp,
            tc.tile_pool(name="otp", bufs=2) as otp,          # OT / L16 / lr32
            tc.tile_pool(name="rp", bufs=1) as rp,            # a2a_out chunks
            tc.tile_pool(name="ypp", bufs=2) as ypp,          # outproj partials
            tc.tile_pool(name="dram", bufs=1, space="DRAM") as dram,
            tc.tile_pool(name="pers", bufs=1) as pers,
        ):
            # ---- persistent SBUF ----
            KT = [pers.tile([128, T], f16, tag=f"KT{p}", name=f"KT{p}")
                  for p in range(2)]
            # per-head Q, zero-padded to K=128 so scores run full-array mode
            # (row-tiled K=64 scores force a TensorE drain on every switch
            # to/from the 128-row P@V/proj matmuls — measured net loss)
            QT = [pers.tile([128, T], f16, tag=f"QT{h}", name=f"QT{h}")
                  for h in range(HPC)]
            V = pers.tile([128, 16, HPC, HD + 1], f16, tag="Vsb", name="Vsb")
            onesf = pers.tile([128, HD], f16, tag="onesf", name="onesf")
            bo_sb = pers.tile([128, 8], f32, tag="bo_sb", name="bo_sb")
            wk_sb = pers.tile([128, 8, HN], f16, tag="wk_sb", name="wk_sb")
            wq_sb = pers.tile([128, 8, HN], f16, tag="wq_sb", name="wq_sb")
            wv_sb = pers.tile([128, 8, HN], f16, tag="wv_sb", name="wv_sb")

            nc.sync.dma_start(onesf[:], onesv_d.ap())
            # ones column of the V operand via memset (a DMA from the inline
            # tensor would issue 8192 two-byte descriptors and starve the
            # lead-in DMA queues)
            nc.vector.memset(V[:, :, :, HD:HD + 1], 1.0)
            # zero the unused K-half of each per-head Q operand
            for h in range(HPC):
                z0, z1 = ((HD, 128) if h % 2 == 0 else (0, HD))
                nc.vector.memset(QT[h][z0:z1, :], 0.0)
            for dd in range(8):
                nc.sync.dma_start(
                    bo_sb[:, dd:dd + 1], bo[dd * 128:(dd + 1) * 128, 0:1]
                )
            nc.sync.dma_start(
                wk_sb[:], wkT[:].rearrange("(c p) n -> p c n", p=128))

            # tiny dummy AllToAll first: pays the mesh-algorithm pipeline
            # fill (~12us/rank staged wave) under the lead-in/attention
            dmy_in = dram.tile([8, 16, 64], f16, name="dmy_in")
            dmy_out = dram.tile([8, 16, 64], f16, name="dmy_out")
            nc.sync.dma_start(
                dmy_in[:].rearrange("s p n -> (s p) n"), onesv_d.ap().bitcast(f16)
            )
            nc.gpsimd.collective_compute(
                "AllToAll",
                mybir.AluOpType.bypass,
                replica_groups=[list(range(N_CORES))],
                ins=[dmy_in.opt()],
                outs=[dmy_out.opt()],
            )

            a2a_in = [
                dram.tile([8, 128, TS], f16, name=f"a2a_in{p}") for p in range(2)
            ]
            a2a_out = [
                dram.tile([8, 128, TS], f16, name=f"a2a_out{p}") for p in range(2)
            ]

            # ---- input staging: x chunks [128, 1024] (kk, col-half) ----
            xs = {}

            def stage_x(xdram, key, kk, ch):
                t0 = bigp.tile([128, 1024], f16, tag="big", name=f"x{key}{kk}{ch}")
                nc.sync.dma_start(t0[:], xdram[kk * 128:(kk + 1) * 128,
                                               ch * 1024:(ch + 1) * 1024])
                xs[(key, kk, ch)] = t0

            def xsl(key, kk, cols):
                # slice [128, 512-ish] of staged chunk for given col range
                ch, off = cols // 1024, cols % 1024
                return xs[(key, kk, ch)][:, off:off + 512]

            # lead-in critical DMAs, in dependency-deadline order: K-proj
            # (wk + xk ch0) gates the first scores, then Q, then V
            for kk in range(8):
                stage_x(xkT, "k", kk, 0)
            nc.sync.dma_start(
                wq_sb[:], wqT[:].rearrange("(c p) n -> p c n", p=128))
            for kk in range(8):
                stage_x(xqT, "q", kk, 0)
            nc.sync.dma_start(
                wv_sb[:], wvT[:].rearrange("(c p) n -> p c n", p=128))
            for kk in range(8):
                stage_x(xvT, "v", kk, 0)

            # ---- projection emitters (JIT-interleaved) ----
            def _proj_ps(w_sb, key, p, cols):
                ps = wkp.tile([128, 512], f32, tag="wk", name="prj")
                for kk in range(8):
                    nc.tensor.matmul(
                        ps[:],
                        w_sb[:, kk, p * 128:(p + 1) * 128],
                        xsl(key, kk, cols),
                        start=(kk == 0), stop=(kk == 7),
                    )
                return ps

            def proj_k(p, cols):
                ps = _proj_ps(wk_sb, "k", p, cols)
                nc.vector.tensor_copy(KT[p][:, cols:cols + 512], ps[:])

            def proj_q(p, cols):
                # pair-packed psum -> per-head padded QT (head parity keeps
                # each head's hn rows at their in-pair partition offsets)
                ps = _proj_ps(wq_sb, "q", p, cols)
                sl = slice(cols, cols + 512)
                nc.vector.tensor_copy(QT[2 * p][0:HD, sl], ps[0:HD, :])
                nc.vector.tensor_copy(QT[2 * p + 1][HD:128, sl], ps[HD:128, :])

            def proj_v(tt):
                # V for key-tile tt (all 4 heads), N=256
                ps = wkp.tile([128, 512], f32, tag="wk", name="pv")
                ch, off = (tt * 128) // 1024, (tt * 128) % 1024
                for kk in range(8):
                    nc.tensor.matmul(
                        ps[:, 0:HN],
                        xs[("v", kk, ch)][:, off:off + 128],
                        wv_sb[:, kk, :],
                        start=(kk == 0), stop=(kk == 7),
                    )
                nc.vector.tensor_copy(
                    V[:, tt, :, 0:HD],
                    ps[:, 0:HN].rearrange("p (h n) -> p h n", h=HPC),
                )

            # ---- attention ----
            def normalize_a(p, qq, po, h):
                # DVE part: reciprocal + denominators row + OT cast
                lr32 = otp.tile([HD + 1, 512], f32, tag="lr", name="lr32")
                ot = otp.tile([HD, 512], f16, tag=f"ot{h}", name="ot")
                lrow = otp.tile([HD + 1, 512], f16, tag="lrow", name="lrow")
                with nc.allow_low_precision(reason="fp16 datapath by design"):
                    nc.vector.reciprocal_approx_fast(lr32[:], po[0:HD + 1, :])
                    nc.vector.tensor_copy(
                        lrow[HD:HD + 1, :], lr32[HD:HD + 1, :])
                    nc.vector.tensor_copy(ot[:], po[0:HD, :])
                return ot, lrow

            def normalize_b(p, qq, h, ot, lrow):
                # PE broadcast of 1/den + DVE mult + a2a staging
                pb = wkp.tile([HD, 512], f32, tag="wk", name="pb")
                nc.tensor.matmul(
                    pb[:],
                    onesf[HD:HD + 1, 0:HD],
                    lrow[HD:HD + 1, :],
                    start=True, stop=True,
                )
                with nc.allow_low_precision(reason="fp16 datapath by design"):
                    nc.vector.tensor_tensor(
                        ot[:], ot[:], pb[:], op=mybir.AluOpType.mult)
                for rep in (0, 4):
                    nc.sync.dma_start(
                        a2a_in[p][qq + rep, h * HD:(h + 1) * HD, :], ot[:])

            def a2a(p):
                nc.gpsimd.collective_compute(
                    "AllToAll",
                    mybir.AluOpType.bypass,
                    replica_groups=[list(range(N_CORES))],
                    ins=[a2a_in[p].opt()],
                    outs=[a2a_out[p].opt()],
                )

            # ---- output projection helpers ----
            # slot j = 2i+p holds the shard from same-batch peer i, pair p =
            # global hn chunk j, so woT is the plain unmasked Wo^T on every
            # core; the a2a_out block index (i or 4+i) is runtime-computed
            # from partition_id (SPMD-uniform instruction stream)
            wo_sb = {}
            rt_sb = {}
            ypart = {}
            pid_reg = nc.sync.alloc_register("pid_sp")
            nc.sync.reg_load(pid_reg, nc.partition_id_tensor[0:1, 0:1])
            pid_sv = nc.sync.snap(pid_reg, donate=True, min_val=0,
                                  max_val=N_CORES - 1)
            blk_base = (pid_sv >= 4) * 4

            def load_rt(j):
                pp, i = j % 2, j // 2
                t0 = rp.tile([128, TS], f16, tag=f"rt{j}", name=f"rt{j}")
                nc.sync.dma_start(t0[:], a2a_out[pp][blk_base + i])
                rt_sb[j] = t0

            def load_wo(j):
                t0 = bigp.tile([128, 1024], f16, tag="big", name=f"wo{j}")
                nc.sync.dma_start(t0[:], woT[j * 128:(j + 1) * 128, :])
                wo_sb[j] = t0

            def outproj_even(dd):
                ps = wkp.tile([128, 512], f32, tag="wk", name=f"ye{dd}")
                for i, j in enumerate(range(0, 8, 2)):
                    nc.tensor.matmul(
                        ps[:],
                        wo_sb[j][:, dd * 128:(dd + 1) * 128],
                        rt_sb[j][:],
                        start=(i == 0), stop=(i == 3),
                    )
                yp = ypp.tile([128, 512], f32, tag=f"yp{dd}", name=f"yp{dd}")
                nc.vector.tensor_scalar_add(yp[:], ps[:], bo_sb[:, dd:dd + 1])
                ypart[dd] = yp

            def outproj_odd(dd):
                ps = wkp.tile([128, 512], f32, tag="wk", name=f"yo{dd}")
                for i, j in enumerate(range(1, 8, 2)):
                    nc.tensor.matmul(
                        ps[:],
                        wo_sb[j][:, dd * 128:(dd + 1) * 128],
                        rt_sb[j][:],
                        start=(i == 0), stop=(i == 3),
                    )
                y = ypp.tile([128, 512], f32, tag=f"yp{dd}", name=f"y{dd}")
                nc.vector.tensor_tensor(
                    y[:], ypart[dd][:], ps[:], op=mybir.AluOpType.add)
                nc.sync.dma_start(out[dd * 128:(dd + 1) * 128, :], y[:])

            # ---- lead-in projections ----
            proj_k(0, 0)                       # K pair0, keys 0:512
            proj_q(0, 0)                       # Q pair0, q 0:512
            proj_v(0)

            # deferred-work queue, consumed one item per tk slot
            def later_dma():
                for kk in range(8):
                    stage_x(xkT, "k", kk, 1)

            def later_dma2():
                for kk in range(8):
                    stage_x(xvT, "v", kk, 1)

            def later_dma3():
                for kk in range(8):
                    stage_x(xqT, "q", kk, 1)

            # comms-warm drip: a real AllToAll issued after ~60us+ of comms
            # idle pays a ~25us re-establishment cost (vs ~5us warm), so a
            # tiny dummy fires every quarter, time-gated by DMAing the
            # current e tile as its input
            cur_e = [None]
            drip_i = [0]

            def drip():
                a, b = ((dmy_in, dmy_out) if drip_i[0] % 2 == 0
                        else (dmy_out, dmy_in))
                drip_i[0] += 1
                nc.sync.dma_start(
                    a[:].rearrange("s p n -> (s p) n"), cur_e[0][:, 0:64])
                nc.gpsimd.collective_compute(
                    "AllToAll",
                    mybir.AluOpType.bypass,
                    replica_groups=[list(range(N_CORES))],
                    ins=[a.opt()],
                    outs=[b.opt()],
                )

            # build fill schedule: dict (p, qq, tk) -> list of callables
            fill = {}

            def add_fill(p, qq, tk, fn):
                fill.setdefault((p, qq, tk), []).append(fn)

            # (p0, qq0): V-proj JIT + K-pair0 key chunks + DMA continuations
            add_fill(0, 0, 0, later_dma)
            add_fill(0, 0, 0, lambda: proj_k(0, 512))
            for tt in range(1, 16):
                add_fill(0, 0, tt - 1, lambda tt=tt: proj_v(tt))
            add_fill(0, 0, 2, later_dma2)
            add_fill(0, 0, 4, lambda: proj_k(0, 1024))
            add_fill(0, 0, 5, later_dma3)
            add_fill(0, 0, 8, lambda: proj_k(0, 1536))
            add_fill(0, 0, 12, lambda: proj_q(0, 512))
            # (p0, qq1): Q0 rest + K1 start (tk>=4: clear of the deferred
            # normalize broadcasts that hold the work pool around tk2)
            add_fill(0, 1, 4, lambda: proj_q(0, 1024))
            add_fill(0, 1, 7, lambda: proj_k(1, 0))
            add_fill(0, 1, 11, lambda: proj_k(1, 512))
            add_fill(0, 1, 14, lambda: proj_q(0, 1536))
            # (p0, qq2): K1 rest + Q1 start
            add_fill(0, 2, 4, lambda: proj_k(1, 1024))
            add_fill(0, 2, 8, lambda: proj_k(1, 1536))
            add_fill(0, 2, 11, lambda: proj_q(1, 0))
            # comms-warm drips, one per quarter
            for pq in ((0, 1), (0, 2), (0, 3), (1, 0), (1, 1), (1, 2), (1, 3)):
                add_fill(pq[0], pq[1], 9, drip)
            # (p0, qq3): Q1 rest
            add_fill(0, 3, 4, lambda: proj_q(1, 512))
            add_fill(0, 3, 8, lambda: proj_q(1, 1024))
            add_fill(0, 3, 12, lambda: proj_q(1, 1536))
            # (p1, qq0): woT loads; rt evens after a2a(0) emission (tk2)
            for i, j in enumerate(range(0, 8, 2)):
                add_fill(1, 0, 2 * i, lambda j=j: load_wo(j))
                add_fill(1, 0, 3 + i, lambda j=j: load_rt(j))
            for i, j in enumerate(range(1, 8, 2)):
                add_fill(1, 0, 8 + 2 * i, lambda j=j: load_wo(j))
            # (p1, qq1..2): even outproj accumulation into SBUF partials
            for dd in range(8):
                qq, tk = 1 + dd // 4, (dd % 4) * 3 + 4
                add_fill(1, qq, tk, lambda dd=dd: outproj_even(dd))


            # ---- main attention loop ----
            norm_pending = []
            for p in range(2):
                for qq in range(NQ):
                    po = [pop.tile([HD + 1, 512], f32, tag=f"po{h}",
                                   name=f"po{p}{qq}{h}") for h in range(2)]
                    es = {}
                    for tk in range(16):
                        s2q = s2p.tile([128, 1024], f32, tag="s2", name="s2q")
                        for h in range(2):
                            nc.tensor.matmul(
                                s2q[:, h * 512:(h + 1) * 512],
                                KT[p][:, tk * 128:(tk + 1) * 128],
                                QT[2 * p + h][:, qq * 512:(qq + 1) * 512],
                                start=True, stop=True,
                            )
                        e = ep.tile([128, 1024], f16, tag="e", name="e")
                        cur_e[0] = e
                        nc.scalar.activation(e[:], s2q[:], EXP, scale=0.125)
                        # deferred normalize_b of the previous quarter
                        if tk == 2 and norm_pending:
                            for fn in norm_pending:
                                fn()
                            norm_pending.clear()
                        if tk > 0:
                            for h in range(2):
                                nc.tensor.matmul(
                                    po[h][:],
                                    V[:, tk - 1, p * 2 + h, :],
                                    es[tk - 1][:, h * 512:(h + 1) * 512],
                                    start=(tk == 1), stop=False,
                                )
                        es[tk] = e
                        for fn in fill.get((p, qq, tk), []):
                            fn()
                    for h in range(2):
                        nc.tensor.matmul(
                            po[h][:],
                            V[:, 15, p * 2 + h, :],
                            es[15][:, h * 512:(h + 1) * 512],
                            start=False, stop=True,
                        )
                    # normalize: DVE part now (frees po), PE/bcast deferred
                    last = (p == 1 and qq == NQ - 1)
                    for h in range(2):
                        ot, lrow = normalize_a(p, qq, po[h], h)
                        norm_pending.append(
                            lambda p=p, qq=qq, h=h, ot=ot, lrow=lrow:
                            normalize_b(p, qq, h, ot, lrow))
                    if last:
                        for fn in norm_pending:
                            fn()
                        norm_pending.clear()
                # pair-0's last normalize_b is deferred into pair-1 qq0; the
                # collective must be EMITTED after those a2a_in DMAs or Tile
                # would order them behind the collective's read (WAR)
                if p == 0:
                    norm_pending.append(lambda: a2a(0))
                else:
                    a2a(1)

            # ---- tail: odd outproj chunks ----
            for j in range(1, 8, 2):
                load_rt(j)
            for dd in range(8):
                outproj_odd(dd)

    nc.compile()
    return nc


def _shard_inputs(k, q, v, Wk, Wq, Wv, Wo, bo):
    # out-projection: slot j = 2*i + p receives peer i's pair-p rows =
    # global hn chunk j, so every core uses the plain unmasked Wo^T
    woT_full = np.ascontiguousarray(Wo.T).astype(np.float16)  # [hn, d]
    in_maps = []
    for c in range(N_CORES):
        i_b, i_h = c // 4, c % 4
        sl = slice(i_h * HN, (i_h + 1) * HN)
        in_maps.append({
            "xqT": q[i_b].T.astype(np.float16),
            "xkT": k[i_b].T.astype(np.float16),
            "xvT": v[i_b].T.astype(np.float16),
            "wqT": Wq[sl].T.astype(np.float16),
            "wkT": Wk[sl].T.astype(np.float16),
            "wvT": Wv[sl].T.astype(np.float16),
            "woT": woT_full,
            "bo": np.ascontiguousarray(bo.reshape(D, 1)).astype(np.float32),
        })
    return in_maps


def _run(in_maps, **kw):
    global _cached
    if _cached is None:
        _cached = _build()
    return run_bass_kernel_spmd(_cached, in_maps, core_ids=list(range(N_CORES)),
                                **kw)


def kernel(k, q, v, Wk, Wq, Wv, Wo, bo):
    k, q, v = (np.asarray(x, np.float32) for x in (k, q, v))
    Wk, Wq, Wv, Wo, bo = (np.asarray(x, np.float32) for x in (Wk, Wq, Wv, Wo, bo))
    in_maps = _shard_inputs(k, q, v, Wk, Wq, Wv, Wo, bo)
    res = _run(in_maps)
    out = np.empty((B, T, D), np.float32)
    for c in range(N_CORES):
        i_b, i_h = c // 4, c % 4
        out[i_b, i_h * TS:(i_h + 1) * TS, :] = res.results[c]["out"].T
    return out


# revision 40
# speedup vs baseline: 1.1903x; 1.0305x over previous
"""Distributed multi-head attention for TRN2 (8 NeuronCores).

Problem: b=2, t=2048, d=1024, h=16 heads, head_dim=64.
  out = softmax((q Wq^T)(k Wk^T)^T / 8) (v Wv^T) Wo^T + bo   (per head)

Sharding: core c -> batch i_b = c//4, head group i_h = c%4 (4 heads = 256
features). Each core projects Q/K/V for its batch+heads, runs attention,
then an 8-core AllToAll reshards head-major -> time-major so each core
computes the final projection for its 512-row time slice.

The schedule is built around the ScalarE exp stream (the hard floor:
16.8M exp elements/core at 1 elem/cycle/lane = ~130us). Scores for a
head PAIR are computed with two row-tiled K=64 matmuls (tile_position
(0,0)/(64,0) via operand base partitions) into the two banks of one
[128,1024] PSUM tile, so a single ACTIVATE covers both heads' exp for a
512-query quarter. Queries are processed in 512-col quarters so the
per-head P@V accumulators (with the softmax-denominator ones-row, M=65)
need only 1 bank each. PSUM: s2 double-buffer 4 banks + po 2 banks +
2-bank work pool used to JIT the K/Q/V projections and the output
projection inside the attention loop's PE slack.

Device layouts (no on-chip transposes): activations streamed as X^T
[d, t]; K/Q kept transposed, pair-packed [128=2x64 hn, t]; scores as S^T
[t_k, t_q]; softmax denominator via an extra ones-column in the V
operand; output projection computes Y^T [d, t_slice].

Datapath fp16 (host-converted); PSUM accumulation fp32.

The AllToAll runs over all 8 cores (4-core groups are unsupported):
shards are duplicated to both batch groups and the final projection uses
16 virtual hn-chunks whose weights are host-side zero-masked for the
chunks belonging to the other batch (SPMD rank-independence). Even
chunks (pair 0) are pre-accumulated right after the first AllToAll and
stashed in SBUF with the bias folded in; the tail after the second
AllToAll only runs the 8 odd-chunk accumulations + one DVE add each.
"""

import numpy as np

import concourse.bass as bass
import concourse.mybir as mybir
import concourse.tile as tile
from concourse import bacc
from concourse.bass_utils import run_bass_kernel_spmd

N_CORES = 8
B = 2
T = 2048
D = 1024
HEADS = 16
HD = 64
HPC = 4            # heads per core
HN = HPC * HD      # 256 head-features per core
TS = T // 4        # 512 time-slice per core after reshard
NQ = T // 512      # 4 query quarters
f32 = mybir.dt.float32
f16 = mybir.dt.float16
EXP = mybir.ActivationFunctionType.Exp

_cached = None


def _build():
    nc = bacc.Bacc("TRN2", target_bir_lowering=False, debug=False,
                   num_devices=N_CORES)

    xqT = nc.dram_tensor("xqT", [D, T], f16, kind="ExternalInput")
    xkT = nc.dram_tensor("xkT", [D, T], f16, kind="ExternalInput")
    xvT = nc.dram_tensor("xvT", [D, T], f16, kind="ExternalInput")
    wqT = nc.dram_tensor("wqT", [D, HN], f16, kind="ExternalInput")
    wkT = nc.dram_tensor("wkT", [D, HN], f16, kind="ExternalInput")
    wvT = nc.dram_tensor("wvT", [D, HN], f16, kind="ExternalInput")
    woT = nc.dram_tensor("woT", [D, D], f16, kind="ExternalInput")
    bsel = nc.dram_tensor("bsel", [128, 2], f32, kind="ExternalInput")
    bo = nc.dram_tensor("bo", [D, 1], f32, kind="ExternalInput")
    out = nc.dram_tensor("out", [D, TS], f32, kind="ExternalOutput")

    onesv_d = nc.inline_tensor(np.ones((128, 64), np.float16), name="onesv_c")

    with tile.TileContext(nc) as tc:
        with (
            tc.tile_pool(name="bigp", bufs=40) as bigp,       # x chunks / wo
            tc.tile_pool(name="ep", bufs=6) as ep,            # exp outputs
            tc.tile_pool(name="s2p", bufs=2, space="PSUM") as s2p,
            tc.tile_pool(name="pop", bufs=1, space="PSUM") as pop,
            tc.tile_pool(name="wkp", bufs=2, space="PSUM") as wkp,
            tc.tile_pool(name="otp", bufs=2) as otp,          # OT / L16 / lr32
            tc.tile_pool(name="rp", bufs=1) as rp,            # a2a_out chunks
            tc.tile_pool(name="ypp", bufs=2) as ypp,          # outproj partials
            tc.tile_pool(name="dram", bufs=1, space="DRAM") as dram,
            tc.tile_pool(name="pers", bufs=1) as pers,
        ):
            # ---- persistent SBUF ----
            KT = [pers.tile([128, T], f16, tag=f"KT{p}", name=f"KT{p}")
                  for p in range(2)]
            # per-head Q, zero-padded to K=128 so scores run full-array mode
            # (row-tiled K=64 scores force a TensorE drain on every switch
            # to/from the 128-row P@V/proj matmuls — measured net loss)
            QT = [pers.tile([128, T], f16, tag=f"QT{h}", name=f"QT{h}")
                  for h in range(HPC)]
            V = pers.tile([128, 16, HPC, HD + 1], f16, tag="Vsb", name="Vsb")
            onesf = pers.tile([128, HD], f16, tag="onesf", name="onesf")
            bo_sb = pers.tile([128, 8], f32, tag="bo_sb", name="bo_sb")
            bsel_sb = pers.tile([128, 2], f32, tag="bsel_sb", name="bsel_sb")
            wk_sb = pers.tile([128, 8, HN], f16, tag="wk_sb", name="wk_sb")
            wq_sb = pers.tile([128, 8, HN], f16, tag="wq_sb", name="wq_sb")
            wv_sb = pers.tile([128, 8, HN], f16, tag="wv_sb", name="wv_sb")

            # ones column of the V operand via memset (a DMA from the inline
            # tensor would issue 8192 two-byte descriptors and starve the
            # lead-in DMA queues)
            nc.vector.memset(V[:, :, :, HD:HD + 1], 1.0)
            # zero the unused K-half of each per-head Q operand
            for h in range(HPC):
                z0, z1 = ((HD, 128) if h % 2 == 0 else (0, HD))
                nc.vector.memset(QT[h][z0:z1, :], 0.0)
            # wk first: it gates the very first projection
            nc.sync.dma_start(
                wk_sb[:], wkT[:].rearrange("(c p) n -> p c n", p=128))

            # tiny dummy AllToAll first: pays the mesh-algorithm pipeline
            # fill (~12us/rank staged wave) under the lead-in/attention
            dmy_in = dram.tile([8, 16, 64], f16, name="dmy_in")
            dmy_out = dram.tile([8, 16, 64], f16, name="dmy_out")
            nc.sync.dma_start(
                dmy_in[:].rearrange("s p n -> (s p) n"), onesv_d.ap().bitcast(f16)
            )
            nc.gpsimd.collective_compute(
                "AllToAll",
                mybir.AluOpType.bypass,
                replica_groups=[list(range(N_CORES))],
                ins=[dmy_in.opt()],
                outs=[dmy_out.opt()],
            )

            def small_dmas():
                nc.sync.dma_start(onesf[:], onesv_d.ap())
                nc.sync.dma_start(bsel_sb[:], bsel[:])
                for dd in range(8):
                    nc.sync.dma_start(
                        bo_sb[:, dd:dd + 1], bo[dd * 128:(dd + 1) * 128, 0:1]
                    )

            a2a_in = [
                dram.tile([8, 128, TS], f16, name=f"a2a_in{p}") for p in range(2)
            ]
            a2a_out = [
                dram.tile([8, 128, TS], f16, name=f"a2a_out{p}") for p in range(2)
            ]

            # ---- input staging ----
            # first 512 cols of xk/xq in [128, 512] pieces (reader waits are
            # per-DMA-instruction, so smaller first pieces unblock the lead-in
            # projections sooner); everything else in [128, 1024] chunks
            xs = {}

            def stage_x(xdram, key, kk, ch):
                t0 = bigp.tile([128, 1024], f16, tag="big", name=f"x{key}{kk}{ch}")
                nc.sync.dma_start(t0[:], xdram[kk * 128:(kk + 1) * 128,
                                               ch * 1024:(ch + 1) * 1024])
                xs[(key, kk, ch)] = t0

            def stage_piece(xdram, key, kk, half):
                if (key, kk, 0) not in xs:
                    xs[(key, kk, 0)] = bigp.tile(
                        [128, 1024], f16, tag="big", name=f"x{key}{kk}0")
                t0 = xs[(key, kk, 0)]
                nc.sync.dma_start(
                    t0[:, half * 512:(half + 1) * 512],
                    xdram[kk * 128:(kk + 1) * 128, half * 512:(half + 1) * 512])

            def xsl(key, kk, cols):
                # slice [128, 512] of staged chunk for given col range
                ch, off = cols // 1024, cols % 1024
                return xs[(key, kk, ch)][:, off:off + 512]

            # lead-in DMAs in dependency-deadline order: K-proj (wk + xk cols
            # 0:512) gates the first scores; then Q quarter 0, V start, the
            # rest of the first column-halves
            for kk in range(8):
                stage_piece(xkT, "k", kk, 0)
            nc.sync.dma_start(
                wq_sb[:], wqT[:].rearrange("(c p) n -> p c n", p=128))
            for kk in range(8):
                stage_piece(xqT, "q", kk, 0)
            for kk in range(8):
                stage_piece(xkT, "k", kk, 1)
            nc.sync.dma_start(
                wv_sb[:], wvT[:].rearrange("(c p) n -> p c n", p=128))
            for kk in range(8):
                stage_x(xvT, "v", kk, 0)
            for kk in range(8):
                stage_piece(xqT, "q", kk, 1)

            # ---- projection emitters (JIT-interleaved) ----
            def _proj_ps(w_sb, key, p, cols):
                ps = wkp.tile([128, 512], f32, tag="wk", name="prj")
                for kk in range(8):
                    nc.tensor.matmul(
                        ps[:],
                        w_sb[:, kk, p * 128:(p + 1) * 128],
                        xsl(key, kk, cols),
                        start=(kk == 0), stop=(kk == 7),
                    )
                return ps

            def proj_k(p, cols):
                ps = _proj_ps(wk_sb, "k", p, cols)
                nc.vector.tensor_copy(KT[p][:, cols:cols + 512], ps[:])

            def proj_q(p, cols):
                # pair-packed psum -> per-head padded QT (head parity keeps
                # each head's hn rows at their in-pair partition offsets)
                ps = _proj_ps(wq_sb, "q", p, cols)
                sl = slice(cols, cols + 512)
                nc.vector.tensor_copy(QT[2 * p][0:HD, sl], ps[0:HD, :])
                nc.vector.tensor_copy(QT[2 * p + 1][HD:128, sl], ps[HD:128, :])

            def proj_v(tt):
                # V for key-tile tt (all 4 heads), N=256
                ps = wkp.tile([128, 512], f32, tag="wk", name="pv")
                ch, off = (tt * 128) // 1024, (tt * 128) % 1024
                for kk in range(8):
                    nc.tensor.matmul(
                        ps[:, 0:HN],
                        xs[("v", kk, ch)][:, off:off + 128],
                        wv_sb[:, kk, :],
                        start=(kk == 0), stop=(kk == 7),
                    )
                nc.vector.tensor_copy(
                    V[:, tt, :, 0:HD],
                    ps[:, 0:HN].rearrange("p (h n) -> p h n", h=HPC),
                )

            # ---- attention ----
            def normalize_a(p, qq, po, h):
                # DVE part: reciprocal + denominators row + OT cast
                lr32 = otp.tile([HD + 1, 512], f32, tag="lr", name="lr32")
                ot = otp.tile([HD, 512], f16, tag=f"ot{h}", name="ot")
                lrow = otp.tile([HD + 1, 512], f16, tag="lrow", name="lrow")
                with nc.allow_low_precision(reason="fp16 datapath by design"):
                    nc.vector.reciprocal_approx_fast(lr32[:], po[0:HD + 1, :])
                    nc.vector.tensor_copy(
                        lrow[HD:HD + 1, :], lr32[HD:HD + 1, :])
                    nc.vector.tensor_copy(ot[:], po[0:HD, :])
                return ot, lrow

            def normalize_b(p, qq, h, ot, lrow):
                # PE broadcast of 1/den + DVE mult + a2a staging
                pb = wkp.tile([HD, 512], f32, tag="wk", name="pb")
                nc.tensor.matmul(
                    pb[:],
                    onesf[HD:HD + 1, 0:HD],
                    lrow[HD:HD + 1, :],
                    start=True, stop=True,
                )
                with nc.allow_low_precision(reason="fp16 datapath by design"):
                    nc.vector.tensor_tensor(
                        ot[:], ot[:], pb[:], op=mybir.AluOpType.mult)
                for rep in (0, 4):
                    nc.sync.dma_start(
                        a2a_in[p][qq + rep, h * HD:(h + 1) * HD, :], ot[:])

            def a2a(p):
                nc.gpsimd.collective_compute(
                    "AllToAll",
                    mybir.AluOpType.bypass,
                    replica_groups=[list(range(N_CORES))],
                    ins=[a2a_in[p].opt()],
                    outs=[a2a_out[p].opt()],
                )

            # ---- output projection helpers ----
            # slot j = 2i+p holds the shard from same-batch peer i, pair p =
            # global hn chunk j, so woT is the plain unmasked Wo^T on every
            # core. Both batch groups' candidate blocks are loaded statically
            # (dynamic-offset DMAs entangle with the GPSIMD collectives) and
            # the right one is picked with a host-provided 0/1 mask on DVE.
            wo_sb = {}
            rt_sb = {}
            ypart = {}

            def load_rt(j):
                pp, i = j % 2, j // 2
                ta = rp.tile([128, TS], f16, tag=f"rta{j}", name=f"rta{j}")
                tb = rp.tile([128, TS], f16, tag=f"rtb{j}", name=f"rtb{j}")
                nc.sync.dma_start(ta[:], a2a_out[pp][i])
                nc.sync.dma_start(tb[:], a2a_out[pp][4 + i])
                rt = rp.tile([128, TS], f16, tag=f"rt{j}", name=f"rt{j}")
                with nc.allow_low_precision(reason="fp16 datapath by design"):
                    nc.vector.tensor_scalar_mul(rt[:], ta[:], bsel_sb[:, 0:1])
                    nc.vector.scalar_tensor_tensor(
                        rt[:], tb[:], bsel_sb[:, 1:2], rt[:],
                        op0=mybir.AluOpType.mult, op1=mybir.AluOpType.add)
                rt_sb[j] = rt

            def load_wo(j):
                t0 = bigp.tile([128, 1024], f16, tag="big", name=f"wo{j}")
                nc.sync.dma_start(t0[:], woT[j * 128:(j + 1) * 128, :])
                wo_sb[j] = t0

            def outproj_even(dd):
                ps = wkp.tile([128, 512], f32, tag="wk", name=f"ye{dd}")
                for i, j in enumerate(range(0, 8, 2)):
                    nc.tensor.matmul(
                        ps[:],
                        wo_sb[j][:, dd * 128:(dd + 1) * 128],
                        rt_sb[j][:],
                        start=(i == 0), stop=(i == 3),
                    )
                yp = ypp.tile([128, 512], f32, tag=f"yp{dd}", name=f"yp{dd}")
                nc.vector.tensor_scalar_add(yp[:], ps[:], bo_sb[:, dd:dd + 1])
                ypart[dd] = yp

            def outproj_odd(dd):
                ps = wkp.tile([128, 512], f32, tag="wk", name=f"yo{dd}")
                for i, j in enumerate(range(1, 8, 2)):
                    nc.tensor.matmul(
                        ps[:],
                        wo_sb[j][:, dd * 128:(dd + 1) * 128],
                        rt_sb[j][:],
                        start=(i == 0), stop=(i == 3),
                    )
                y = ypp.tile([128, 512], f32, tag=f"yp{dd}", name=f"y{dd}")
                nc.vector.tensor_tensor(
                    y[:], ypart[dd][:], ps[:], op=mybir.AluOpType.add)
                nc.sync.dma_start(out[dd * 128:(dd + 1) * 128, :], y[:])

            # ---- lead-in projections ----
            proj_k(0, 0)                       # K pair0, keys 0:512
            proj_q(0, 0)                       # Q pair0, q 0:512

            # deferred-work queue, consumed one item per tk slot
            def later_dma():
                for kk in range(8):
                    stage_x(xkT, "k", kk, 1)

            def later_dma2():
                for kk in range(8):
                    stage_x(xvT, "v", kk, 1)

            def later_dma3():
                for kk in range(8):
                    stage_x(xqT, "q", kk, 1)

            # comms-warm drip: a real AllToAll issued after ~60us+ of comms
            # idle pays a ~25us re-establishment cost (vs ~5us warm), so a
            # tiny dummy fires every quarter, time-gated by DMAing the
            # current e tile as its input
            cur_e = [None]
            drip_i = [0]

            def drip():
                a, b = ((dmy_in, dmy_out) if drip_i[0] % 2 == 0
                        else (dmy_out, dmy_in))
                drip_i[0] += 1
                nc.sync.dma_start(
                    a[:].rearrange("s p n -> (s p) n"), cur_e[0][:, 0:64])
                nc.gpsimd.collective_compute(
                    "AllToAll",
                    mybir.AluOpType.bypass,
                    replica_groups=[list(range(N_CORES))],
                    ins=[a.opt()],
                    outs=[b.opt()],
                )

            # build fill schedule: dict (p, qq, tk) -> list of callables
            fill = {}

            def add_fill(p, qq, tk, fn):
                fill.setdefault((p, qq, tk), []).append(fn)

            # (p0, qq0): V-proj JIT + K-pair0 key chunks + DMA continuations
            add_fill(0, 0, 0, later_dma)
            add_fill(0, 0, 0, lambda: proj_v(0))
            add_fill(0, 0, 0, lambda: proj_v(1))
            add_fill(0, 0, 1, lambda: proj_k(0, 512))
            add_fill(0, 0, 2, later_dma2)
            for tt in range(2, 16):
                add_fill(0, 0, tt, lambda tt=tt: proj_v(tt))
            add_fill(0, 0, 3, small_dmas)
            add_fill(0, 0, 5, lambda: proj_k(0, 1024))
            add_fill(0, 0, 6, later_dma3)
            add_fill(0, 0, 9, lambda: proj_k(0, 1536))
            add_fill(0, 0, 12, lambda: proj_q(0, 512))
            # (p0, qq1): Q0 rest + K1 start (tk>=4: clear of the deferred
            # normalize broadcasts that hold the work pool around tk2)
            add_fill(0, 1, 4, lambda: proj_q(0, 1024))
            add_fill(0, 1, 7, lambda: proj_k(1, 0))
            add_fill(0, 1, 11, lambda: proj_k(1, 512))
            add_fill(0, 1, 14, lambda: proj_q(0, 1536))
            # (p0, qq2): K1 rest + Q1 start
            add_fill(0, 2, 4, lambda: proj_k(1, 1024))
            add_fill(0, 2, 8, lambda: proj_k(1, 1536))
            add_fill(0, 2, 11, lambda: proj_q(1, 0))
            # comms-warm drips: enough to keep the mesh wave established, but
            # none adjacent to the real collectives (a queued drip delays them)
            for pq, tk in (((0, 1), 9), ((0, 2), 9), ((0, 3), 9),
                           ((1, 1), 9), ((1, 2), 14)):
                add_fill(pq[0], pq[1], tk, drip)
            # (p0, qq3): Q1 rest
            add_fill(0, 3, 4, lambda: proj_q(1, 512))
            add_fill(0, 3, 8, lambda: proj_q(1, 1024))
            add_fill(0, 3, 12, lambda: proj_q(1, 1536))
            # (p1, qq0): woT loads; rt evens after a2a(0) emission (tk2)
            for i, j in enumerate(range(0, 8, 2)):
                add_fill(1, 0, 2 * i, lambda j=j: load_wo(j))
                add_fill(1, 0, 3 + i, lambda j=j: load_rt(j))
            for i, j in enumerate(range(1, 8, 2)):
                add_fill(1, 0, 8 + 2 * i, lambda j=j: load_wo(j))
            # (p1, qq1..2): even outproj accumulation into SBUF partials
            for dd in range(8):
                qq, tk = 1 + dd // 4, (dd % 4) * 3 + 4
                add_fill(1, qq, tk, lambda dd=dd: outproj_even(dd))


            # ---- main attention loop ----
            norm_pending = []
            for p in range(2):
                for qq in range(NQ):
                    po = [pop.tile([HD + 1, 512], f32, tag=f"po{h}",
                                   name=f"po{p}{qq}{h}") for h in range(2)]
                    es = {}
                    for tk in range(16):
                        s2q = s2p.tile([128, 1024], f32, tag="s2", name="s2q")
                        for h in range(2):
                            nc.tensor.matmul(
                                s2q[:, h * 512:(h + 1) * 512],
                                KT[p][:, tk * 128:(tk + 1) * 128],
                                QT[2 * p + h][:, qq * 512:(qq + 1) * 512],
                                start=True, stop=True,
                            )
                        e = ep.tile([128, 1024], f16, tag="e", name="e")
                        cur_e[0] = e
                        nc.scalar.activation(e[:], s2q[:], EXP, scale=0.125)
                        # deferred normalize_b of the previous quarter
                        if tk == 2 and norm_pending:
                            for fn in norm_pending:
                                fn()
                            norm_pending.clear()
                        if tk > 0:
                            for h in range(2):
                                nc.tensor.matmul(
                                    po[h][:],
                                    V[:, tk - 1, p * 2 + h, :],
                                    es[tk - 1][:, h * 512:(h + 1) * 512],
                                    start=(tk == 1), stop=False,
                                )
                        es[tk] = e
                        for fn in fill.get((p, qq, tk), []):
                            fn()
                    for h in range(2):
                        nc.tensor.matmul(
                            po[h][:],
                            V[:, 15, p * 2 + h, :],
                            es[15][:, h * 512:(h + 1) * 512],
                            start=False, stop=True,
                        )
                    # normalize: DVE part now (frees po), PE/bcast deferred
                    last = (p == 1 and qq == NQ - 1)
                    for h in range(2):
                        ot, lrow = normalize_a(p, qq, po[h], h)
                        norm_pending.append(
                            lambda p=p, qq=qq, h=h, ot=ot, lrow=lrow:
                            normalize_b(p, qq, h, ot, lrow))
                    if last:
                        for fn in norm_pending:
                            fn()
                        norm_pending.clear()
                # pair-0's last normalize_b is deferred into pair-1 qq0; the
                # collective must be EMITTED after those a2a_in DMAs or Tile
                # would order them behind the collective's read (WAR)
                if p == 0:
                    norm_pending.append(lambda: a2a(0))
                else:
                    a2a(1)

            # ---- tail: odd outproj chunks ----
            for j in range(1, 8, 2):
                load_rt(j)
            for dd in range(8):
                outproj_odd(dd)

    nc.compile()
    return nc


def _shard_inputs(k, q, v, Wk, Wq, Wv, Wo, bo):
    # out-projection: slot j = 2*i + p receives peer i's pair-p rows =
    # global hn chunk j, so every core uses the plain unmasked Wo^T
    woT_full = np.ascontiguousarray(Wo.T).astype(np.float16)  # [hn, d]
    in_maps = []
    for c in range(N_CORES):
        i_b, i_h = c // 4, c % 4
        sl = slice(i_h * HN, (i_h + 1) * HN)
        in_maps.append({
            "xqT": q[i_b].T.astype(np.float16),
            "xkT": k[i_b].T.astype(np.float16),
            "xvT": v[i_b].T.astype(np.float16),
            "wqT": Wq[sl].T.astype(np.float16),
            "wkT": Wk[sl].T.astype(np.float16),
            "wvT": Wv[sl].T.astype(np.float16),
            "woT": woT_full,
            "bsel": np.broadcast_to(
                np.array([[1.0, 0.0]] if i_b == 0 else [[0.0, 1.0]],
                         np.float32), (128, 2)).copy(),
            "bo": np.ascontiguousarray(bo.reshape(D, 1)).astype(np.float32),
        })
    return in_maps


def _run(in_maps, **kw):
    global _cached
    if _cached is None:
        _cached = _build()
    return run_bass_kernel_spmd(_cached, in_maps, core_ids=list(range(N_CORES)),
                                **kw)


def kernel(k, q, v, Wk, Wq, Wv, Wo, bo):
    k, q, v = (np.asarray(x, np.float32) for x in (k, q, v))
    Wk, Wq, Wv, Wo, bo = (np.asarray(x, np.float32) for x in (Wk, Wq, Wv, Wo, bo))
    in_maps = _shard_inputs(k, q, v, Wk, Wq, Wv, Wo, bo)
    res = _run(in_maps)
    out = np.empty((B, T, D), np.float32)
    for c in range(N_CORES):
        i_b, i_h = c // 4, c % 4
        out[i_b, i_h * TS:(i_h + 1) * TS, :] = res.results[c]["out"].T
    return out


# revision 47
# speedup vs baseline: 1.2625x; 1.0606x over previous
"""Distributed multi-head attention for TRN2 (8 NeuronCores).

Problem: b=2, t=2048, d=1024, h=16 heads, head_dim=64.
  out = softmax((q Wq^T)(k Wk^T)^T / 8) (v Wv^T) Wo^T + bo   (per head)

Sharding: core c -> batch i_b = c//4, head group i_h = c%4 (4 heads = 256
features). Each core projects Q/K/V for its batch+heads, runs attention,
then an 8-core AllToAll reshards head-major -> time-major so each core
computes the final projection for its 512-row time slice.

The schedule is built around the ScalarE exp stream (the hard floor:
16.8M exp elements/core at 1 elem/cycle/lane = ~130us). Scores for a
head PAIR are computed with two row-tiled K=64 matmuls (tile_position
(0,0)/(64,0) via operand base partitions) into the two banks of one
[128,1024] PSUM tile, so a single ACTIVATE covers both heads' exp for a
512-query quarter. Queries are processed in 512-col quarters so the
per-head P@V accumulators (with the softmax-denominator ones-row, M=65)
need only 1 bank each. PSUM: s2 double-buffer 4 banks + po 2 banks +
2-bank work pool used to JIT the K/Q/V projections and the output
projection inside the attention loop's PE slack.

Device layouts (no on-chip transposes): activations streamed as X^T
[d, t]; K/Q kept transposed, pair-packed [128=2x64 hn, t]; scores as S^T
[t_k, t_q]; softmax denominator via an extra ones-column in the V
operand; output projection computes Y^T [d, t_slice].

Datapath fp16 (host-converted); PSUM accumulation fp32.

The AllToAll runs over all 8 cores (4-core groups are unsupported):
shards are duplicated to both batch groups and the final projection uses
16 virtual hn-chunks whose weights are host-side zero-masked for the
chunks belonging to the other batch (SPMD rank-independence). Even
chunks (pair 0) are pre-accumulated right after the first AllToAll and
stashed in SBUF with the bias folded in; the tail after the second
AllToAll only runs the 8 odd-chunk accumulations + one DVE add each.
"""

import numpy as np

import concourse.bass as bass
import concourse.mybir as mybir
import concourse.tile as tile
from concourse import bacc
from concourse.bass_utils import run_bass_kernel_spmd

N_CORES = 8
B = 2
T = 2048
D = 1024
HEADS = 16
HD = 64
HPC = 4            # heads per core
HN = HPC * HD      # 256 head-features per core
TS = T // 4        # 512 time-slice per core after reshard
NQ = T // 512      # 4 query quarters
f32 = mybir.dt.float32
f16 = mybir.dt.float16
EXP = mybir.ActivationFunctionType.Exp

_cached = None


def _build():
    nc = bacc.Bacc("TRN2", target_bir_lowering=False, debug=False,
                   num_devices=N_CORES)

    xqT = nc.dram_tensor("xqT", [D, T], f16, kind="ExternalInput")
    xkT = nc.dram_tensor("xkT", [D, T], f16, kind="ExternalInput")
    xvT = nc.dram_tensor("xvT", [D, T], f16, kind="ExternalInput")
    wqT = nc.dram_tensor("wqT", [D, HN], f16, kind="ExternalInput")
    wkT = nc.dram_tensor("wkT", [D, HN], f16, kind="ExternalInput")
    wvT = nc.dram_tensor("wvT", [D, HN], f16, kind="ExternalInput")
    woT = nc.dram_tensor("woT", [D, D], f16, kind="ExternalInput")
    bsel = nc.dram_tensor("bsel", [128, 2], f32, kind="ExternalInput")
    bo = nc.dram_tensor("bo", [D, 1], f32, kind="ExternalInput")
    out = nc.dram_tensor("out", [D, TS], f32, kind="ExternalOutput")

    onesv_d = nc.inline_tensor(np.ones((128, 64), np.float16), name="onesv_c")

    with tile.TileContext(nc) as tc:
        with (
            tc.tile_pool(name="bigp", bufs=40) as bigp,       # x chunks / wo
            tc.tile_pool(name="ep", bufs=6) as ep,            # exp outputs
            tc.tile_pool(name="s2p", bufs=2, space="PSUM") as s2p,
            tc.tile_pool(name="pop", bufs=1, space="PSUM") as pop,
            tc.tile_pool(name="wkp", bufs=2, space="PSUM") as wkp,
            tc.tile_pool(name="otp", bufs=2) as otp,          # OT / L16 / lr32
            tc.tile_pool(name="rp", bufs=1) as rp,            # a2a_out chunks
            tc.tile_pool(name="ypp", bufs=2) as ypp,          # outproj partials
            tc.tile_pool(name="dram", bufs=1, space="DRAM") as dram,
            tc.tile_pool(name="pers", bufs=1) as pers,
        ):
            # ---- persistent SBUF ----
            KT = [pers.tile([128, T], f16, tag=f"KT{p}", name=f"KT{p}")
                  for p in range(2)]
            # per-head Q, zero-padded to K=128 so scores run full-array mode
            # (row-tiled K=64 scores force a TensorE drain on every switch
            # to/from the 128-row P@V/proj matmuls — measured net loss)
            QT = [pers.tile([128, T], f16, tag=f"QT{h}", name=f"QT{h}")
                  for h in range(HPC)]
            V = pers.tile([128, 16, HPC, HD + 1], f16, tag="Vsb", name="Vsb")
            onesf = pers.tile([128, HD], f16, tag="onesf", name="onesf")
            bo_sb = pers.tile([128, 8], f32, tag="bo_sb", name="bo_sb")
            bsel_sb = pers.tile([128, 2], f32, tag="bsel_sb", name="bsel_sb")
            wk_sb = pers.tile([128, 8, HN], f16, tag="wk_sb", name="wk_sb")
            wq_sb = pers.tile([128, 8, HN], f16, tag="wq_sb", name="wq_sb")
            wv_sb = pers.tile([128, 8, HN], f16, tag="wv_sb", name="wv_sb")

            # ones column of the V operand via memset (a DMA from the inline
            # tensor would issue 8192 two-byte descriptors and starve the
            # lead-in DMA queues)
            nc.vector.memset(V[:, :, :, HD:HD + 1], 1.0)
            # zero the unused K-half of each per-head Q operand
            for h in range(HPC):
                z0, z1 = ((HD, 128) if h % 2 == 0 else (0, HD))
                nc.vector.memset(QT[h][z0:z1, :], 0.0)
            # wk first: it gates the very first projection
            nc.sync.dma_start(
                wk_sb[:], wkT[:].rearrange("(c p) n -> p c n", p=128))

            # tiny dummy AllToAll first: pays the mesh-algorithm pipeline
            # fill (~12us/rank staged wave) under the lead-in/attention
            dmy_in = dram.tile([8, 16, 64], f16, name="dmy_in")
            dmy_out = dram.tile([8, 16, 64], f16, name="dmy_out")
            nc.sync.dma_start(
                dmy_in[:].rearrange("s p n -> (s p) n"), onesv_d.ap().bitcast(f16)
            )
            nc.gpsimd.collective_compute(
                "AllToAll",
                mybir.AluOpType.bypass,
                replica_groups=[list(range(N_CORES))],
                ins=[dmy_in.opt()],
                outs=[dmy_out.opt()],
            )

            def small_dmas():
                nc.sync.dma_start(onesf[:], onesv_d.ap())
                nc.sync.dma_start(bsel_sb[:], bsel[:])
                for dd in range(8):
                    nc.sync.dma_start(
                        bo_sb[:, dd:dd + 1], bo[dd * 128:(dd + 1) * 128, 0:1]
                    )

            a2a_in = [
                dram.tile([8, 128, TS], f16, name=f"a2a_in{p}") for p in range(2)
            ]
            a2a_out = [
                dram.tile([8, 128, TS], f16, name=f"a2a_out{p}") for p in range(2)
            ]

            # ---- input staging ----
            # first 512 cols of xk/xq in [128, 512] pieces (reader waits are
            # per-DMA-instruction, so smaller first pieces unblock the lead-in
            # projections sooner); everything else in [128, 1024] chunks
            xs = {}

            def stage_x(xdram, key, kk, ch):
                t0 = bigp.tile([128, 1024], f16, tag="big", name=f"x{key}{kk}{ch}")
                nc.sync.dma_start(t0[:], xdram[kk * 128:(kk + 1) * 128,
                                               ch * 1024:(ch + 1) * 1024])
                xs[(key, kk, ch)] = t0

            def stage_piece(xdram, key, kk, half):
                if (key, kk, 0) not in xs:
                    xs[(key, kk, 0)] = bigp.tile(
                        [128, 1024], f16, tag="big", name=f"x{key}{kk}0")
                t0 = xs[(key, kk, 0)]
                nc.sync.dma_start(
                    t0[:, half * 512:(half + 1) * 512],
                    xdram[kk * 128:(kk + 1) * 128, half * 512:(half + 1) * 512])

            def xsl(key, kk, cols):
                # slice [128, 512] of staged chunk for given col range
                ch, off = cols // 1024, cols % 1024
                return xs[(key, kk, ch)][:, off:off + 512]

            # lead-in DMAs in dependency-deadline order: K-proj (wk + xk cols
            # 0:512) gates the first scores; then Q quarter 0, V keys 0:512
            # (V-proj JIT feeds the lagged first-quarter P@V), then the rest
            for kk in range(8):
                stage_piece(xkT, "k", kk, 0)
            nc.sync.dma_start(
                wq_sb[:], wqT[:].rearrange("(c p) n -> p c n", p=128))
            for kk in range(8):
                stage_piece(xqT, "q", kk, 0)
            nc.sync.dma_start(
                wv_sb[:], wvT[:].rearrange("(c p) n -> p c n", p=128))
            for kk in range(8):
                stage_piece(xvT, "v", kk, 0)
            for kk in range(8):
                stage_piece(xkT, "k", kk, 1)
            for kk in range(8):
                stage_piece(xvT, "v", kk, 1)
            for kk in range(8):
                stage_piece(xqT, "q", kk, 1)

            # ---- projection emitters (JIT-interleaved) ----
            def _proj_ps(w_sb, key, p, cols):
                ps = wkp.tile([128, 512], f32, tag="wk", name="prj")
                for kk in range(8):
                    nc.tensor.matmul(
                        ps[:],
                        w_sb[:, kk, p * 128:(p + 1) * 128],
                        xsl(key, kk, cols),
                        start=(kk == 0), stop=(kk == 7),
                    )
                return ps

            def proj_k(p, cols):
                ps = _proj_ps(wk_sb, "k", p, cols)
                nc.vector.tensor_copy(KT[p][:, cols:cols + 512], ps[:])

            def proj_q(p, cols):
                # pair-packed psum -> per-head padded QT (head parity keeps
                # each head's hn rows at their in-pair partition offsets)
                ps = _proj_ps(wq_sb, "q", p, cols)
                sl = slice(cols, cols + 512)
                nc.vector.tensor_copy(QT[2 * p][0:HD, sl], ps[0:HD, :])
                nc.vector.tensor_copy(QT[2 * p + 1][HD:128, sl], ps[HD:128, :])

            def proj_v(tt):
                # V for key-tile tt (all 4 heads), N=256
                ps = wkp.tile([128, 512], f32, tag="wk", name="pv")
                ch, off = (tt * 128) // 1024, (tt * 128) % 1024
                for kk in range(8):
                    nc.tensor.matmul(
                        ps[:, 0:HN],
                        xs[("v", kk, ch)][:, off:off + 128],
                        wv_sb[:, kk, :],
                        start=(kk == 0), stop=(kk == 7),
                    )
                nc.vector.tensor_copy(
                    V[:, tt, :, 0:HD],
                    ps[:, 0:HN].rearrange("p (h n) -> p h n", h=HPC),
                )

            # ---- attention ----
            def normalize_a(p, qq, po, h):
                # DVE part: reciprocal + denominators row + OT cast
                lr32 = otp.tile([HD + 1, 512], f32, tag="lr", name="lr32")
                ot = otp.tile([HD, 512], f16, tag=f"ot{h}", name="ot")
                lrow = otp.tile([HD + 1, 512], f16, tag="lrow", name="lrow")
                with nc.allow_low_precision(reason="fp16 datapath by design"):
                    nc.vector.reciprocal_approx_fast(lr32[:], po[0:HD + 1, :])
                    nc.vector.tensor_copy(
                        lrow[HD:HD + 1, :], lr32[HD:HD + 1, :])
                    nc.vector.tensor_copy(ot[:], po[0:HD, :])
                return ot, lrow

            def normalize_b(p, qq, h, ot, lrow, last=False):
                # broadcast 1/den across the 64 hd partitions, then DVE mult
                # + a2a staging. The broadcast normally round-trips through
                # DRAM (partition-stride-0 DMA read) to keep it off the PE;
                # the final quarter uses the PE ones-matmul instead since the
                # DMA hop latency would sit on the a2a(1) critical path.
                if last:
                    pb = wkp.tile([HD, 512], f32, tag="wk", name="pb")
                    nc.tensor.matmul(
                        pb[:],
                        onesf[HD:HD + 1, 0:HD],
                        lrow[HD:HD + 1, :],
                        start=True, stop=True,
                    )
                else:
                    row_d = dram.tile([1, 512], f16, name=f"row{p}{qq}{h}")
                    nc.sync.dma_start(row_d[:], lrow[HD:HD + 1, :])
                    pb = otp.tile([HD, 512], f16, tag="pbb", name="pbb")
                    nc.sync.dma_start(pb[:], row_d[:].partition_broadcast(HD))
                with nc.allow_low_precision(reason="fp16 datapath by design"):
                    nc.vector.tensor_tensor(
                        ot[:], ot[:], pb[:], op=mybir.AluOpType.mult)
                for rep in (0, 4):
                    nc.sync.dma_start(
                        a2a_in[p][qq + rep, h * HD:(h + 1) * HD, :], ot[:])

            def a2a(p):
                nc.gpsimd.collective_compute(
                    "AllToAll",
                    mybir.AluOpType.bypass,
                    replica_groups=[list(range(N_CORES))],
                    ins=[a2a_in[p].opt()],
                    outs=[a2a_out[p].opt()],
                )

            # ---- output projection helpers ----
            # slot j = 2i+p holds the shard from same-batch peer i, pair p =
            # global hn chunk j, so woT is the plain unmasked Wo^T on every
            # core. Both batch groups' candidate blocks are loaded statically
            # (dynamic-offset DMAs entangle with the GPSIMD collectives) and
            # the right one is picked with a host-provided 0/1 mask on DVE.
            wo_sb = {}
            rt_sb = {}
            ypart = {}

            def load_rt(j):
                pp, i = j % 2, j // 2
                ta = rp.tile([128, TS], f16, tag=f"rta{j}", name=f"rta{j}")
                tb = rp.tile([128, TS], f16, tag=f"rtb{j}", name=f"rtb{j}")
                nc.sync.dma_start(ta[:], a2a_out[pp][i])
                nc.sync.dma_start(tb[:], a2a_out[pp][4 + i])
                rt = rp.tile([128, TS], f16, tag=f"rt{j}", name=f"rt{j}")
                with nc.allow_low_precision(reason="fp16 datapath by design"):
                    nc.vector.tensor_scalar_mul(rt[:], ta[:], bsel_sb[:, 0:1])
                    nc.vector.scalar_tensor_tensor(
                        rt[:], tb[:], bsel_sb[:, 1:2], rt[:],
                        op0=mybir.AluOpType.mult, op1=mybir.AluOpType.add)
                rt_sb[j] = rt

            def load_wo(j):
                t0 = bigp.tile([128, 1024], f16, tag="big", name=f"wo{j}")
                nc.sync.dma_start(t0[:], woT[j * 128:(j + 1) * 128, :])
                wo_sb[j] = t0

            def outproj_even(dd):
                ps = wkp.tile([128, 512], f32, tag="wk", name=f"ye{dd}")
                for i, j in enumerate(range(0, 8, 2)):
                    nc.tensor.matmul(
                        ps[:],
                        wo_sb[j][:, dd * 128:(dd + 1) * 128],
                        rt_sb[j][:],
                        start=(i == 0), stop=(i == 3),
                    )
                yp = ypp.tile([128, 512], f32, tag=f"yp{dd}", name=f"yp{dd}")
                nc.vector.tensor_scalar_add(yp[:], ps[:], bo_sb[:, dd:dd + 1])
                ypart[dd] = yp

            def outproj_odd(dd):
                ps = wkp.tile([128, 512], f32, tag="wk", name=f"yo{dd}")
                for i, j in enumerate(range(1, 8, 2)):
                    nc.tensor.matmul(
                        ps[:],
                        wo_sb[j][:, dd * 128:(dd + 1) * 128],
                        rt_sb[j][:],
                        start=(i == 0), stop=(i == 3),
                    )
                y = ypp.tile([128, 512], f32, tag=f"yp{dd}", name=f"y{dd}")
                nc.vector.tensor_tensor(
                    y[:], ypart[dd][:], ps[:], op=mybir.AluOpType.add)
                nc.sync.dma_start(out[dd * 128:(dd + 1) * 128, :], y[:])

            # ---- lead-in projections ----
            proj_k(0, 0)                       # K pair0, keys 0:512
            proj_q(0, 0)                       # Q pair0, q 0:512

            # deferred-work queue, consumed one item per tk slot
            def later_dma():
                for kk in range(8):
                    stage_x(xkT, "k", kk, 1)

            def later_dma2():
                for kk in range(8):
                    stage_x(xvT, "v", kk, 1)

            def later_dma3():
                for kk in range(8):
                    stage_x(xqT, "q", kk, 1)

            # comms-warm drip: a real AllToAll issued after ~60us+ of comms
            # idle pays a ~25us re-establishment cost (vs ~5us warm), so a
            # tiny dummy fires every quarter, time-gated by DMAing the
            # current e tile as its input
            cur_e = [None]
            drip_i = [0]

            def drip():
                a, b = ((dmy_in, dmy_out) if drip_i[0] % 2 == 0
                        else (dmy_out, dmy_in))
                drip_i[0] += 1
                nc.sync.dma_start(
                    a[:].rearrange("s p n -> (s p) n"), cur_e[0][:, 0:64])
                nc.gpsimd.collective_compute(
                    "AllToAll",
                    mybir.AluOpType.bypass,
                    replica_groups=[list(range(N_CORES))],
                    ins=[a.opt()],
                    outs=[b.opt()],
                )

            # build fill schedule: dict (p, qq, tk) -> list of callables
            fill = {}

            def add_fill(p, qq, tk, fn):
                fill.setdefault((p, qq, tk), []).append(fn)

            # (p0, qq0): V-proj JIT + K-pair0 key chunks + DMA continuations;
            # P@V lags 3 tks here so the V(tt) JIT never blocks the in-order
            # PE stream while xv streams in
            add_fill(0, 0, 0, later_dma)
            add_fill(0, 0, 1, lambda: proj_k(0, 512))
            add_fill(0, 0, 1, lambda: proj_v(0))
            add_fill(0, 0, 2, later_dma2)
            add_fill(0, 0, 2, lambda: proj_v(1))
            add_fill(0, 0, 3, small_dmas)
            add_fill(0, 0, 3, lambda: proj_v(2))
            add_fill(0, 0, 4, lambda: proj_v(3))
            add_fill(0, 0, 5, lambda: proj_k(0, 1024))
            add_fill(0, 0, 5, lambda: proj_v(4))
            add_fill(0, 0, 6, later_dma3)
            add_fill(0, 0, 6, lambda: proj_v(5))
            add_fill(0, 0, 7, lambda: proj_v(6))
            add_fill(0, 0, 7, lambda: proj_v(7))
            add_fill(0, 0, 8, lambda: proj_v(8))
            add_fill(0, 0, 9, lambda: proj_k(0, 1536))
            add_fill(0, 0, 9, lambda: proj_v(9))
            for tt in range(10, 16):
                add_fill(0, 0, tt, lambda tt=tt: proj_v(tt))
            add_fill(0, 0, 12, lambda: proj_q(0, 512))
            # (p0, qq1): Q0 rest + K1 start (tk>=4: clear of the deferred
            # normalize broadcasts that hold the work pool around tk2)
            add_fill(0, 1, 4, lambda: proj_q(0, 1024))
            add_fill(0, 1, 7, lambda: proj_k(1, 0))
            add_fill(0, 1, 11, lambda: proj_k(1, 512))
            add_fill(0, 1, 14, lambda: proj_q(0, 1536))
            # (p0, qq2): K1 rest + Q1 start
            add_fill(0, 2, 4, lambda: proj_k(1, 1024))
            add_fill(0, 2, 8, lambda: proj_k(1, 1536))
            add_fill(0, 2, 11, lambda: proj_q(1, 0))
            # comms-warm drips: enough to keep the mesh wave established, but
            # none adjacent to the real collectives (a queued drip delays them)
            for pq, tk in (((0, 1), 9), ((0, 2), 9), ((0, 3), 9),
                           ((1, 1), 9), ((1, 2), 9), ((1, 3), 4)):
                add_fill(pq[0], pq[1], tk, drip)
            # (p0, qq3): Q1 rest
            add_fill(0, 3, 4, lambda: proj_q(1, 512))
            add_fill(0, 3, 8, lambda: proj_q(1, 1024))
            add_fill(0, 3, 12, lambda: proj_q(1, 1536))
            # (p1, qq0): woT loads; rt evens after a2a(0) emission (tk2)
            for i, j in enumerate(range(0, 8, 2)):
                add_fill(1, 0, 2 * i, lambda j=j: load_wo(j))
                add_fill(1, 0, 3 + i, lambda j=j: load_rt(j))
            for i, j in enumerate(range(1, 8, 2)):
                add_fill(1, 0, 8 + 2 * i, lambda j=j: load_wo(j))
            # (p1, qq1..2): even outproj accumulation into SBUF partials
            for dd in range(8):
                qq, tk = 1 + dd // 4, (dd % 4) * 3 + 4
                add_fill(1, qq, tk, lambda dd=dd: outproj_even(dd))


            # ---- main attention loop ----
            norm_pending = []
            for p in range(2):
                for qq in range(NQ):
                    po = [pop.tile([HD + 1, 512], f32, tag=f"po{h}",
                                   name=f"po{p}{qq}{h}") for h in range(2)]
                    es = {}
                    lag = 3 if (p == 0 and qq == 0) else 1
                    for tk in range(16):
                        s2q = s2p.tile([128, 1024], f32, tag="s2", name="s2q")
                        for h in range(2):
                            nc.tensor.matmul(
                                s2q[:, h * 512:(h + 1) * 512],
                                KT[p][:, tk * 128:(tk + 1) * 128],
                                QT[2 * p + h][:, qq * 512:(qq + 1) * 512],
                                start=True, stop=True,
                            )
                        e = ep.tile([128, 1024], f16, tag="e", name="e")
                        cur_e[0] = e
                        nc.scalar.activation(e[:], s2q[:], EXP, scale=0.125)
                        # deferred normalize_b of the previous quarter
                        if tk == 2 and norm_pending:
                            for fn in norm_pending:
                                fn()
                            norm_pending.clear()
                        if tk >= lag:
                            src = tk - lag
                            for h in range(2):
                                nc.tensor.matmul(
                                    po[h][:],
                                    V[:, src, p * 2 + h, :],
                                    es[src][:, h * 512:(h + 1) * 512],
                                    start=(src == 0), stop=False,
                                )
                        es[tk] = e
                        for fn in fill.get((p, qq, tk), []):
                            fn()
                    for src in range(16 - lag, 16):
                        for h in range(2):
                            nc.tensor.matmul(
                                po[h][:],
                                V[:, src, p * 2 + h, :],
                                es[src][:, h * 512:(h + 1) * 512],
                                start=False, stop=(src == 15),
                            )
                    # normalize: DVE part now (frees po), PE/bcast deferred
                    last = (p == 1 and qq == NQ - 1)
                    for h in range(2):
                        ot, lrow = normalize_a(p, qq, po[h], h)
                        norm_pending.append(
                            lambda p=p, qq=qq, h=h, ot=ot, lrow=lrow, lst=last:
                            normalize_b(p, qq, h, ot, lrow, last=lst))
                    if last:
                        for fn in norm_pending:
                            fn()
                        norm_pending.clear()
                # pair-0's last normalize_b is deferred into pair-1 qq0; the
                # collective must be EMITTED after those a2a_in DMAs or Tile
                # would order them behind the collective's read (WAR)
                if p == 0:
                    norm_pending.append(lambda: a2a(0))
                else:
                    a2a(1)

            # ---- tail: odd outproj chunks ----
            for j in range(1, 8, 2):
                load_rt(j)
            for dd in range(8):
                outproj_odd(dd)

    nc.compile()
    return nc


def _shard_inputs(k, q, v, Wk, Wq, Wv, Wo, bo):
    # out-projection: slot j = 2*i + p receives peer i's pair-p rows =
    # global hn chunk j, so every core uses the plain unmasked Wo^T
    woT_full = np.ascontiguousarray(Wo.T).astype(np.float16)  # [hn, d]
    in_maps = []
    for c in range(N_CORES):
        i_b, i_h = c // 4, c % 4
        sl = slice(i_h * HN, (i_h + 1) * HN)
        in_maps.append({
            "xqT": q[i_b].T.astype(np.float16),
            "xkT": k[i_b].T.astype(np.float16),
            "xvT": v[i_b].T.astype(np.float16),
            "wqT": Wq[sl].T.astype(np.float16),
            "wkT": Wk[sl].T.astype(np.float16),
            "wvT": Wv[sl].T.astype(np.float16),
            "woT": woT_full,
            "bsel": np.broadcast_to(
                np.array([[1.0, 0.0]] if i_b == 0 else [[0.0, 1.0]],
                         np.float32), (128, 2)).copy(),
            "bo": np.ascontiguousarray(bo.reshape(D, 1)).astype(np.float32),
        })
    return in_maps


def _run(in_maps, **kw):
    global _cached
    if _cached is None:
        _cached = _build()
    return run_bass_kernel_spmd(_cached, in_maps, core_ids=list(range(N_CORES)),
                                **kw)


def kernel(k, q, v, Wk, Wq, Wv, Wo, bo):
    k, q, v = (np.asarray(x, np.float32) for x in (k, q, v))
    Wk, Wq, Wv, Wo, bo = (np.asarray(x, np.float32) for x in (Wk, Wq, Wv, Wo, bo))
    in_maps = _shard_inputs(k, q, v, Wk, Wq, Wv, Wo, bo)
    res = _run(in_maps)
    out = np.empty((B, T, D), np.float32)
    for c in range(N_CORES):
        i_b, i_h = c // 4, c % 4
        out[i_b, i_h * TS:(i_h + 1) * TS, :] = res.results[c]["out"].T
    return out
